# revision 22
# baseline (speedup 1.0000x reference)
"""AttnGraphSAGE on 8 Trainium2 NeuronCores (Bass/Tile) — v2.

Math restructuring (unchanged from v1): attention logits depend only on the
SOURCE node, so the whole edge phase is ONE segment-sum over dst of per-src
rows G[n] = [E_0*x_jm_0 (64) | E_1*x_jm_1 (64) | E_0 | E_1] (130 values).

v2 performance changes:
  * G rows are bf16, 256-elem / 512B strides (was f32 768B): halves the
    random-gather HBM traffic and the AllGather volume.  All matmul operands
    (weights, activations, indicator) are bf16 -> 1-pass PE instead of 4.
  * dma_gather calls are PREPARE_ONLY + trigger_dma: GpSimd only generates
    descriptors (~1us/call) instead of blocking until the DMA lands
    (~7us/call serialized in v1).  DMA queues stay deep and overlap compute.
  * The G table is AllGather'd in TWO halves (A = rows [0,3072) of each
    core, B = rows [3072,6250)): AG(A) overlaps phase-A compute of the B
    rows, and each half has <32768 rows so the two gather address ranges
    double as the int16-index split (no separate lo/hi split needed).
  * Exact per-block index counts (padded only to the max across the 8 cores
    so the program stays SPMD-uniform), 0-padded: no trailing -1 indices,
    ~15% fewer descriptors than v1's global-max padding.
"""
import os
import sys
import types
import hashlib
import contextlib

sys.path.insert(0, "/opt/trn_rl_repo")

import numpy as np
import ml_dtypes

import concourse.bass as bass
import concourse.bacc as bacc
import concourse.mybir as mybir
from concourse import tile

# ---------------------------------------------------------------- constants
N = 50000
E = 800000
IN = 128
F = 64
H = 2
N_CORES = 8
NC_N = N // N_CORES          # 6250 nodes per core
BLK = 128                    # dst nodes per block
ROW = 256                    # G row stride in bf16 elems (512B)
GVAL = 2 * F + H             # 130 used cols
ASPLIT = 3072                # rows per core in AllGather half A
BSPLIT = NC_N - ASPLIT       # 3178 rows in half B
CHUNK = 512                  # phase-A node chunk (6 chunks cover ASPLIT)
CAP = int(os.environ.get("GNN_CAP", "3500"))   # max idxs per gather call
F32 = mybir.dt.float32
BF16 = mybir.dt.bfloat16
I16 = mybir.dt.int16
AF = mybir.ActivationFunctionType
OP = mybir.AluOpType
BN_EPS = 1e-5
LEAKY = 0.2
BF = ml_dtypes.bfloat16


# ------------------------------------------------------- axon profile shim
def _install_hookshim():
    if "antenv.axon_hooks" in sys.modules:
        return
    mod = types.ModuleType("antenv.axon_hooks")
    _h = [None]
    mod.set_axon_ntff_profile_hook = lambda h: _h.__setitem__(0, h)
    mod.get_axon_ntff_profile_hook = lambda: _h[0]
    try:
        import antenv
        sys.modules["antenv.axon_hooks"] = mod
        antenv.axon_hooks = mod
        from trn_agent_boot.trn_boot import _ntff_profile_via_ctypes
        mod.set_axon_ntff_profile_hook(
            _ntff_profile_via_ctypes("/opt/axon/libaxon_pjrt.so")
        )
    except Exception:
        pass


def remap_dmasw_waits(nc):
    """Remap waits on Tile's DMASW lane semaphores to the per-queue gather
    DMA-completion sems.

    Tile assigned each PREPARE_ONLY gather prep a DMASW lane (round-robin)
    and derived all downstream waits (consumers, ring flow control) as
    ``DMASW{lane} >= 16*tick``.  But the sem actually baked into the
    descriptors (and bumped by the SDMA engines) is our per-queue gdma sem,
    so those lane sems never move.  Each prep records its assigned
    (lane proc, tick); since each queue's ring is FIFO, the k-th prep of
    queue q has completed exactly when gdma{q} >= 16*k.  Rewrite every
    DMASW wait for (lane, tick) into the equivalent (and race-free)
    per-queue wait."""
    from concourse.tile_sem_assignment import PROC_NAME_TO_IDX
    inv_proc = {v: k for k, v in PROC_NAME_TO_IDX.items()}

    insts = []
    for func in nc.m.functions:
        for block in func.blocks:
            insts.extend(block.instructions)

    # (lane_name, 16*tick) -> (gdma sem id, gdma name, block-level target)
    lane_map = {}
    for inst in insts:
        if type(inst).__name__ == "InstDMAGatherAnt" and \
                getattr(inst, "gen_mode", 0) == 1:
            lane = inv_proc[inst.bass_scheduled_proc]
            upd = inst.sync_info.on_update[0]
            assert upd.ant_name.startswith("gdma"), upd.ant_name
            key = (lane, 16 * inst.bass_scheduled_tick)
            assert key not in lane_map, key
            lane_map[key] = (upd.id, upd.ant_name,
                             nc._gnn_prep_targets[inst.name])

    n = 0
    for inst in insts:
        si = inst.sync_info
        if not (si and si.on_wait):
            continue
        changed = False
        new_waits = []
        for w in si.on_wait:
            if w.ant_name and w.ant_name.startswith("DMASW"):
                lane = w.ant_name.split("_")[0]
                sid, sname, thresh = lane_map[(lane, w.wait_value)]
                new_waits.append(mybir.SyncWait(
                    sync_type="semaphore", id=sid, wait_mode="sem-ge-imm",
                    wait_value=thresh, ant_name=sname))
                changed = True
            else:
                new_waits.append(w)
        if changed:
            si.on_wait = new_waits
            n += 1
    return n


# ------------------------------------------------------------ wait legalize
def legalize_waits(nc):
    """TRN2 TPB instructions have ONE sync-wait slot (EventSemaphore has 2);
    hoist extra waits left by the Tile scheduler into EVSEM prequels."""
    n_fixed = 0
    for func in nc.m.functions:
        for block in func.blocks:
            new_insts = []
            for inst in block.instructions:
                si = inst.sync_info
                waits = list(si.on_wait) if si and si.on_wait else []
                cap = 2 if isinstance(inst, mybir.InstEventSemaphore) else 1
                if isinstance(inst, mybir.InstDrain):
                    cap = 1
                if len(waits) > cap:
                    extra, keep = waits[:-cap], waits[-cap:]
                    for i in range(0, len(extra), 2):
                        new_insts.append(
                            mybir.InstEventSemaphore(
                                name=nc.get_next_instruction_name(),
                                ins=[],
                                outs=[],
                                engine=inst.engine,
                                sync_info=mybir.SyncInfo(
                                    on_wait=extra[i:i + 2], on_update=[]
                                ),
                            )
                        )
                    si.on_wait = keep
                    n_fixed += 1
                new_insts.append(inst)
            block.instructions[:] = new_insts
    return n_fixed


# ----------------------------------------------------------- host preprocess
def preprocess(edge_index):
    """Sort edges by dst, partition per core / per 128-dst block, split each
    block's edges into A/B-region runs (by source row within its owner core),
    pad counts to the per-block max across cores (program is SPMD-uniform),
    and build the int16 index planes + bf16 dst-local planes."""
    nb = (NC_N + BLK - 1) // BLK
    src = np.asarray(edge_index[0], np.int64)
    dst = np.asarray(edge_index[1], np.int64)
    order = np.argsort(dst, kind="stable")
    ds, ss = dst[order], src[order]

    core = ds // NC_N
    blk = (ds - core * NC_N) // BLK
    gblk = core * nb + blk
    n_gblk = N_CORES * nb
    bbounds = np.searchsorted(gblk, np.arange(n_gblk + 1))

    # source slot within the AllGather'd table halves
    sc = ss // NC_N
    r = ss - sc * NC_N
    in_a = r < ASPLIT
    slot = np.where(in_a, sc * ASPLIT + r, sc * BSPLIT + (r - ASPLIT))

    runs = {}    # (core, block) -> (a_slots, a_dl, b_slots, b_dl)
    n_a = np.zeros((N_CORES, nb), np.int64)
    n_b = np.zeros((N_CORES, nb), np.int64)
    for g in range(n_gblk):
        e0, e1 = bbounds[g], bbounds[g + 1]
        c, b = g // nb, g % nb
        base = c * NC_N + b * BLK
        sl, dl, ia = slot[e0:e1], ds[e0:e1] - base, in_a[e0:e1]
        a_s, a_d = sl[ia], dl[ia]
        b_s, b_d = sl[~ia], dl[~ia]
        # ascending slot order inside each run -> ascending HBM addresses
        oa, ob = np.argsort(a_s, kind="stable"), np.argsort(b_s, kind="stable")
        runs[(c, b)] = (a_s[oa], a_d[oa], b_s[ob], b_d[ob])
        n_a[c, b], n_b[c, b] = len(a_s), len(b_s)

    n_a_u = n_a.max(axis=0)          # uniform (max-over-cores) counts
    n_b_u = n_b.max(axis=0)

    # per-block call layout (same for every core)
    def split_calls(n):
        if n == 0:
            return []
        k = (n + CAP - 1) // CAP
        szs = [n // k + (1 if i < n % k else 0) for i in range(k)]
        # round each call except the last up to a 128 multiple so calls
        # start on subtile boundaries of the stage tile
        out = []
        rem = n
        for i in range(k - 1):
            s = (szs[i] + 127) // 128 * 128
            out.append(s)
            rem -= s
        out.append(rem)
        return out

    blocks = []                      # per block: dict of layout info
    w_idx = 0
    tot_s = 0
    for b in range(nb):
        ca = split_calls(int(n_a_u[b]))
        cb = split_calls(int(n_b_u[b]))
        s_a = (int(n_a_u[b]) + BLK - 1) // BLK
        s_b_ = (int(n_b_u[b]) + BLK - 1) // BLK
        calls = []
        col = w_idx
        sub = 0
        for cs in ca:
            w = (cs + 15) // 16
            calls.append(("A", cs, col, sub))
            col += w
            sub += (cs + 127) // 128
        assert sub == s_a
        for cs in cb:
            w = (cs + 15) // 16
            calls.append(("B", cs, col, sub))
            col += w
            sub += (cs + 127) // 128
        assert sub == s_a + s_b_
        blocks.append(dict(n_a=int(n_a_u[b]), n_b=int(n_b_u[b]),
                           s=s_a + s_b_, s_a=s_a, calls=calls,
                           dl_off=tot_s))
        w_idx = col
        tot_s += s_a + s_b_

    idx_dev = np.zeros((N_CORES, 16, w_idx), np.int16)
    dl_dev = np.full((N_CORES, BLK, tot_s), -1.0, np.float32)

    def wrap16(vals, n_uni):
        # pad with valid dummy index 0 up to the uniform count
        a = np.zeros(((n_uni + 15) // 16 * 16,), np.int64)
        a[:len(vals)] = vals
        return a.reshape(-1, 16).T.astype(np.int16)

    for c in range(N_CORES):
        for b in range(nb):
            bl = blocks[b]
            a_s, a_d, b_s, b_d = runs[(c, b)]
            # index plane: A calls then B calls, contiguous columns
            awrap = wrap16(a_s, bl["n_a"])
            bwrap = wrap16(b_s, bl["n_b"])
            c0 = bl["calls"][0][2]
            idx_dev[c, :, c0:c0 + awrap.shape[1]] = awrap
            idx_dev[c, :, c0 + awrap.shape[1]:
                    c0 + awrap.shape[1] + bwrap.shape[1]] = bwrap
            # dst-local plane: slot k of the stage -> (p=k%128, s=k//128)
            dcol = np.full((bl["s"] * BLK,), -1.0, np.float32)
            dcol[:len(a_d)] = a_d
            dcol[bl["s_a"] * BLK:bl["s_a"] * BLK + len(b_d)] = b_d
            dl_dev[c, :, bl["dl_off"]:bl["dl_off"] + bl["s"]] = \
                dcol.reshape(bl["s"], BLK).T

    idx_full = np.tile(idx_dev, (1, 8, 1))     # replicate to 128 partitions
    s_max = max(bl["s"] for bl in blocks)
    meta = dict(nb=nb, blocks=blocks, w_idx=w_idx, tot_s=tot_s, s_max=s_max)
    return idx_full, dl_dev.astype(BF), meta


def pack_weights(inp):
    """Host-side packing of the small replicated weight tensors (bf16)."""
    def bd(av):  # [H, 2F] -> block-diag [H*F, H] halves (query, msg)
        av = np.asarray(av, np.float32)
        q = np.zeros((H * F, H), np.float32)
        m = np.zeros((H * F, H), np.float32)
        for h in range(H):
            q[h * F:(h + 1) * F, h] = av[h, :F]
            m[h * F:(h + 1) * F, h] = av[h, F:]
        return q, m

    w = {}
    for l in (0, 1):
        w[f"Wr{l}"] = np.asarray(inp[f"Wr{l}"], np.float32).astype(BF)
        w[f"Wn{l}"] = np.asarray(inp[f"Wn{l}"], np.float32).astype(BF)
        w[f"Wa{l}"] = np.asarray(inp[f"Wa{l}"], np.float32).astype(BF)
        q_, m_ = bd(inp[f"av{l}"])
        w[f"avq{l}"], w[f"avm{l}"] = q_.astype(BF), m_.astype(BF)
        w[f"bn{l}"] = np.stack(
            [np.asarray(inp[f"g{l}"], np.float32),
             np.asarray(inp[f"b{l}"], np.float32)], axis=1)  # [64,2] f32
    w["headW"] = np.asarray(inp["head_W"], np.float32).astype(BF)
    w["headb"] = np.asarray(inp["head_b"], np.float32).reshape(3, 1)
    w["iota"] = np.broadcast_to(np.arange(BLK, dtype=np.float32),
                                (BLK, BLK)).astype(BF)
    w["identb"] = np.eye(BLK, dtype=np.float32).astype(BF)
    w["identf"] = np.eye(BLK, dtype=np.float32)
    bo = np.zeros((H, H * F), np.float32)
    for h in range(H):
        bo[h, h * F:(h + 1) * F] = 1.0
    w["blkones"] = bo.astype(BF)
    return w


# ------------------------------------------------------------ device program
def build_program(meta):
    nb = meta["nb"]
    blocks = meta["blocks"]
    w_idx = meta["w_idx"]
    tot_s = meta["tot_s"]
    s_max = meta["s_max"]
    dims = [IN, F]

    nc = bacc.Bacc(None, num_swdge_queues=4)
    nc._gnn_prep_targets = {}   # prep inst name -> block-level gdma target

    # ---- I/O
    xT = nc.declare_dram_parameter("xT", [IN, NC_N], BF16, isOutput=False)
    idx_in = nc.declare_dram_parameter("idx", [BLK, w_idx], I16, isOutput=False)
    dl_in = nc.declare_dram_parameter("dstloc", [BLK, tot_s], BF16, isOutput=False)
    wext = {}
    for l in (0, 1):
        d = dims[l]
        wext[f"Wr{l}"] = nc.declare_dram_parameter(f"Wr{l}", [d, F], BF16, isOutput=False)
        wext[f"Wn{l}"] = nc.declare_dram_parameter(f"Wn{l}", [d, H * F], BF16, isOutput=False)
        wext[f"Wa{l}"] = nc.declare_dram_parameter(f"Wa{l}", [d, H * F], BF16, isOutput=False)
        wext[f"avq{l}"] = nc.declare_dram_parameter(f"avq{l}", [H * F, H], BF16, isOutput=False)
        wext[f"avm{l}"] = nc.declare_dram_parameter(f"avm{l}", [H * F, H], BF16, isOutput=False)
        wext[f"bn{l}"] = nc.declare_dram_parameter(f"bn{l}", [F, 2], F32, isOutput=False)
    wext["headW"] = nc.declare_dram_parameter("headW", [F, 3], BF16, isOutput=False)
    wext["headb"] = nc.declare_dram_parameter("headb", [3, 1], F32, isOutput=False)
    wext["iota"] = nc.declare_dram_parameter("iota", [BLK, BLK], BF16, isOutput=False)
    wext["identb"] = nc.declare_dram_parameter("identb", [BLK, BLK], BF16, isOutput=False)
    wext["identf"] = nc.declare_dram_parameter("identf", [BLK, BLK], F32, isOutput=False)
    wext["blkones"] = nc.declare_dram_parameter("blkones", [H, H * F], BF16, isOutput=False)
    out_ext = nc.declare_dram_parameter("out", [3, NC_N], F32, isOutput=True)

    # ---- internal DRAM
    g_src = [nc.dram_tensor(f"g_src{l}", [NC_N, ROW], BF16) for l in (0, 1)]
    g_fullA = [nc.dram_tensor(f"g_fullA{l}", [N_CORES * ASPLIT, ROW], BF16,
                              addr_space="Shared") for l in (0, 1)]
    g_fullB = [nc.dram_tensor(f"g_fullB{l}", [N_CORES * BSPLIT, ROW], BF16,
                              addr_space="Shared") for l in (0, 1)]
    bn_src = [nc.dram_tensor(f"bn_src{l}", [F, 2], F32) for l in (0, 1)]
    bn_out = [nc.dram_tensor(f"bn_out{l}", [F, 2], F32, addr_space="Shared")
              for l in (0, 1)]
    groups = [list(range(N_CORES))]

    n_chunks = (NC_N + CHUNK - 1) // CHUNK
    a_chunks = ASPLIT // CHUNK       # chunks covering the A half exactly
    stage_cap = int(os.environ.get("GNN_STAGE", "9"))
    layer_cap = int(os.environ.get("GNN_LAYERS", "2"))

    dma_sems = [nc.alloc_semaphore(f"gdma{q}") for q in range(4)]

    with tile.TileContext(nc) as tc:
        with contextlib.ExitStack() as ctx:
            cpool = ctx.enter_context(tc.tile_pool(name="const", bufs=1))
            wp = ctx.enter_context(tc.tile_pool(name="work", bufs=2))
            hp = ctx.enter_context(tc.tile_pool(name="resid", bufs=1))
            pp = ctx.enter_context(tc.tile_pool(name="psA", bufs=1, space="PSUM"))
            pb = ctx.enter_context(tc.tile_pool(name="psB", bufs=2, space="PSUM"))

            # ---- load constants
            wsb = {}
            for k, ext in wext.items():
                t = cpool.tile(list(ext.shape), ext.dtype, tag=k)
                nc.sync.dma_start(out=t[:], in_=ext[:])
                wsb[k] = t
            idx_sb = cpool.tile([BLK, w_idx], I16, tag="idx")
            nc.sync.dma_start(out=idx_sb[:], in_=idx_in[:])
            dl_sb = cpool.tile([BLK, tot_s], BF16, tag="dl")
            nc.sync.dma_start(out=dl_sb[:], in_=dl_in[:])

            hT_res = hp.tile([F, NC_N], F32, tag="hres")
            hT_act = hp.tile([F, NC_N], BF16, tag="hact")
            nc.vector.memset(hT_act[:], 0.0)
            scr = hp.tile([F, (NC_N + 1) // 2], F32, tag="scr")
            stats = hp.tile([F, 6], F32, tag="stats")
            bnsc = hp.tile([F, 8], F32, tag="bnsc")

            for l in (0, 1)[:layer_cap]:
                d = dims[l]
                # ================= phase A: per-node G rows + x_root =======
                for ci in range(n_chunks):
                    c0 = ci * CHUNK
                    cw = min(CHUNK, NC_N - c0)
                    if l == 0:
                        rhs = wp.tile([IN, CHUNK], BF16, tag="xchunk")
                        nc.sync.dma_start(out=rhs[:, :cw], in_=xT[:, c0:c0 + cw])
                        rhs_ap = rhs[:IN, :cw]
                    else:
                        rhs_ap = hT_act[:F, c0:c0 + cw]

                    ps_jm = pp.tile([H * F, CHUNK], F32, tag="jm", space="PSUM")
                    ps_iq = pp.tile([H * F, CHUNK], F32, tag="iq", space="PSUM")
                    ps_r = pp.tile([F, CHUNK], F32, tag="r", space="PSUM")
                    nc.tensor.matmul(out=ps_jm[:, :cw], lhsT=wsb[f"Wn{l}"][:d, :],
                                     rhs=rhs_ap, start=True, stop=True)
                    nc.tensor.matmul(out=ps_iq[:, :cw], lhsT=wsb[f"Wa{l}"][:d, :],
                                     rhs=rhs_ap, start=True, stop=True)
                    nc.tensor.matmul(out=ps_r[:, :cw], lhsT=wsb[f"Wr{l}"][:d, :],
                                     rhs=rhs_ap, start=True, stop=True)
                    nc.vector.tensor_copy(hT_res[:, c0:c0 + cw], ps_r[:, :cw])

                    jm = wp.tile([H * F, CHUNK], BF16, tag="jm_sb")
                    nc.vector.tensor_copy(jm[:, :cw], ps_jm[:, :cw])
                    # leaky(x) = max(x, 0.2x)
                    lkjm = wp.tile([H * F, CHUNK], BF16, tag="lkjm")
                    nc.scalar.mul(lkjm[:, :cw], ps_jm[:, :cw], LEAKY)
                    nc.vector.tensor_tensor(out=lkjm[:, :cw], in0=lkjm[:, :cw],
                                            in1=jm[:, :cw], op=OP.max)
                    iq = wp.tile([H * F, CHUNK], BF16, tag="iq_sb")
                    nc.vector.tensor_copy(iq[:, :cw], ps_iq[:, :cw])
                    lkiq = wp.tile([H * F, CHUNK], BF16, tag="lkiq")
                    nc.scalar.mul(lkiq[:, :cw], ps_iq[:, :cw], LEAKY)
                    nc.vector.tensor_tensor(out=lkiq[:, :cw], in0=lkiq[:, :cw],
                                            in1=iq[:, :cw], op=OP.max)
                    ps_s = pp.tile([H, CHUNK], F32, tag="s", space="PSUM")
                    nc.tensor.matmul(out=ps_s[:, :cw], lhsT=wsb[f"avq{l}"][:],
                                     rhs=lkiq[:, :cw], start=True, stop=False)
                    nc.tensor.matmul(out=ps_s[:, :cw], lhsT=wsb[f"avm{l}"][:],
                                     rhs=lkjm[:, :cw], start=False, stop=True)
                    e_sb = wp.tile([H, CHUNK], BF16, tag="esb")
                    nc.scalar.activation(e_sb[:, :cw], ps_s[:, :cw], AF.Exp)
                    # broadcast E over the per-head 64 features via matmul
                    ps_eb = pp.tile([H * F, CHUNK], F32, tag="iq", space="PSUM")
                    nc.tensor.matmul(out=ps_eb[:, :cw], lhsT=wsb["blkones"][:],
                                     rhs=e_sb[:, :cw], start=True, stop=True)
                    eb = wp.tile([H * F, CHUNK], BF16, tag="eb")
                    nc.vector.tensor_copy(eb[:, :cw], ps_eb[:, :cw])
                    y = wp.tile([H * F, CHUNK], BF16, tag="y")
                    nc.vector.tensor_tensor(out=y[:, :cw], in0=jm[:, :cw],
                                            in1=eb[:, :cw], op=OP.mult)
                    # write G rows (transpose to node-major)
                    for q in range(0, cw, BLK):
                        qw = min(BLK, cw - q)
                        ps_t = pb.tile([BLK, BLK], BF16, tag="tp", space="PSUM")
                        nc.tensor.transpose(out=ps_t[:qw, :], in_=y[:, q:q + qw],
                                            identity=wsb["identb"][:])
                        ps_e = pb.tile([BLK, BLK], BF16, tag="tp", space="PSUM")
                        nc.tensor.transpose(out=ps_e[:qw, :H], in_=e_sb[:, q:q + qw],
                                            identity=wsb["identb"][:H, :H])
                        gt = wp.tile([BLK, ROW], BF16, tag="gt")
                        nc.vector.tensor_copy(gt[:qw, 0:H * F], ps_t[:qw, :])
                        nc.vector.tensor_copy(gt[:qw, H * F:GVAL], ps_e[:qw, :H])
                        nc.sync.dma_start(
                            out=g_src[l][c0 + q:c0 + q + qw, :],
                            in_=gt[:qw, :])
                    # AllGather half A as soon as its rows are written
                    if ci == a_chunks - 1 and stage_cap >= 2:
                        nc.gpsimd.collective_compute(
                            "AllGather", OP.bypass, replica_groups=groups,
                            ins=[g_src[l][0:ASPLIT, :]], outs=[g_fullA[l][:]])

                if stage_cap < 2:
                    continue
                nc.gpsimd.collective_compute(
                    "AllGather", OP.bypass, replica_groups=groups,
                    ins=[g_src[l][ASPLIT:NC_N, :]], outs=[g_fullB[l][:]])

                # ================= phase B: gather + indicator matmul ======
                if stage_cap < 3:
                    continue
                if l == 0:
                    cum_calls = [0, 0, 0, 0]
                    prev_trigger = [None, None, None, None]
                for b in range(nb):
                    bl = blocks[b]
                    b0 = b * BLK
                    bw = min(BLK, NC_N - b0)
                    s_b = bl["s"]
                    qnum = b % 4
                    stage = wp.tile([BLK, s_max, ROW], BF16, tag="stage", bufs=3)
                    if b < 3 and l == 0:
                        nc.vector.memset(stage[:], 0.0)
                    sync_mode = bool(os.environ.get("GNN_SYNC"))
                    gate = None
                    if not sync_mode and cum_calls[qnum] > 0:
                        # drain queue q's previous block before pushing more:
                        # bounds triggered-incomplete work to ONE block per
                        # queue, which makes the cumulative 16-per-call
                        # thresholds exact (no DMA-engine skew hazard)
                        gate = nc.gpsimd.wait_ge(dma_sems[qnum],
                                                 16 * cum_calls[qnum])
                        deps = bass.InstructionNameOrderedSet()
                        deps.add(prev_trigger[qnum].ins.name)
                        gate.ins.add_nosync_dependencies_from(deps)
                    blk_preps = []
                    for reg, cs, col, sub in bl["calls"]:
                        in_view = (g_fullA[l][:] if reg == "A"
                                   else g_fullB[l][:])
                        nsub = (cs + 127) // 128
                        if sync_mode:
                            nc.gpsimd.dma_gather(
                                out_ap=stage[:, sub:sub + nsub, :],
                                in_ap=in_view,
                                idxs_ap=idx_sb[:, col:col + (cs + 15) // 16],
                                num_idxs=cs, num_idxs_reg=cs,
                                elem_size=ROW, queue_num=qnum)
                        else:
                            p = nc.gpsimd.dma_gather(
                                out_ap=stage[:, sub:sub + nsub, :],
                                in_ap=in_view,
                                idxs_ap=idx_sb[:, col:col + (cs + 15) // 16],
                                num_idxs=cs, num_idxs_reg=cs,
                                elem_size=ROW, queue_num=qnum,
                                prepare_only=True, sem=dma_sems[qnum])
                            cum_calls[qnum] += 1
                            blk_preps.append(p)
                            # keep per-queue ring order == program order:
                            # a prep may not be scheduled before the previous
                            # trigger of its queue (else that trigger fires
                            # this prep's descriptors)
                            deps = bass.InstructionNameOrderedSet()
                            have = False
                            if gate is not None:
                                deps.add(gate.ins.name)
                                have = True
                            if prev_trigger[qnum] is not None:
                                deps.add(prev_trigger[qnum].ins.name)
                                have = True
                            if have:
                                p.ins.add_nosync_dependencies_from(deps)
                    if not sync_mode:
                        prev_trigger[qnum] = nc.gpsimd.trigger_dma(
                            count=None, queue_num=qnum)
                        # consumers must wait for the whole block's calls
                        for p in blk_preps:
                            nc._gnn_prep_targets[p.ins.name] = \
                                16 * cum_calls[qnum]
                    if stage_cap < 4:
                        continue
                    off = bl["dl_off"]
                    ind = wp.tile([BLK, s_max * BLK], BF16, tag="ind", bufs=3)
                    nc.vector.tensor_tensor(
                        out=ind[:, 0:s_b * BLK].rearrange("p (s i) -> p s i", i=BLK),
                        in0=dl_sb[:, off:off + s_b][:, :, None]
                            .to_broadcast([BLK, s_b, BLK]),
                        in1=wsb["iota"][:, None, :].to_broadcast([BLK, s_b, BLK]),
                        op=OP.is_equal)
                    ps_blk = pb.tile([BLK, GVAL], F32, tag="blk", space="PSUM")
                    for j in range(s_b):
                        mm = nc.tensor.matmul(out=ps_blk[:],
                                              lhsT=ind[:, j * BLK:(j + 1) * BLK],
                                              rhs=stage[:, j, 0:GVAL],
                                              start=(j == 0), stop=(j == s_b - 1))
                        if not sync_mode and mm is not None:
                            # scheduling-order (no-sync) edge: keep the
                            # stage consumers after this block's trigger in
                            # the PE stream, else PE head-of-line blocks on
                            # data whose trigger hasn't dispatched yet
                            deps = bass.InstructionNameOrderedSet()
                            deps.add(prev_trigger[qnum].ins.name)
                            mm.ins.add_nosync_dependencies_from(deps)
                    sb = wp.tile([BLK, GVAL], F32, tag="sbblk")
                    nc.vector.tensor_copy(sb[:], ps_blk[:])
                    rec = wp.tile([BLK, H], F32, tag="rec")
                    nc.vector.tensor_scalar_add(rec[:], sb[:, H * F:GVAL], 1e-30)
                    nc.vector.reciprocal(rec[:], rec[:])
                    agg = wp.tile([BLK, F], F32, tag="agg")
                    tmp = wp.tile([BLK, F], F32, tag="tmp")
                    nc.vector.tensor_tensor(out=agg[:], in0=sb[:, 0:F],
                                            in1=rec[:, 0:1].to_broadcast([BLK, F]),
                                            op=OP.mult)
                    nc.vector.tensor_tensor(out=tmp[:], in0=sb[:, F:2 * F],
                                            in1=rec[:, 1:2].to_broadcast([BLK, F]),
                                            op=OP.mult)
                    nc.vector.tensor_add(out=agg[:], in0=agg[:], in1=tmp[:])
                    agg_bf = wp.tile([BLK, F], BF16, tag="aggbf")
                    nc.vector.tensor_copy(agg_bf[:], agg[:])
                    ps_t = pb.tile([BLK, BLK], BF16, tag="tp", space="PSUM")
                    nc.tensor.transpose(out=ps_t[:F, :], in_=agg_bf[:, :F],
                                        identity=wsb["identb"][:])
                    nc.vector.tensor_add(out=hT_res[:, b0:b0 + bw],
                                         in0=hT_res[:, b0:b0 + bw],
                                         in1=ps_t[:F, :bw])

                # ================= BatchNorm + ReLU ========================
                if stage_cap < 5:
                    continue
                nc.vector.reduce_sum(out=stats[:, 0:1], in_=hT_res[:, 0:NC_N],
                                     axis=mybir.AxisListType.X)
                half = (NC_N + 1) // 2
                nc.scalar.square(scr[:, 0:half], hT_res[:, 0:half])
                nc.vector.reduce_sum(out=stats[:, 1:2], in_=scr[:, 0:half],
                                     axis=mybir.AxisListType.X)
                nc.scalar.square(scr[:, 0:NC_N - half], hT_res[:, half:NC_N])
                nc.vector.reduce_sum(out=stats[:, 4:5], in_=scr[:, 0:NC_N - half],
                                     axis=mybir.AxisListType.X)
                nc.vector.tensor_add(out=stats[:, 1:2], in0=stats[:, 1:2],
                                     in1=stats[:, 4:5])
                nc.sync.dma_start(out=bn_src[l][:], in_=stats[:, 0:2])
                nc.gpsimd.collective_compute(
                    "AllReduce", OP.add, replica_groups=groups,
                    ins=[bn_src[l][:]], outs=[bn_out[l][:]])
                nc.sync.dma_start(out=stats[:, 2:4], in_=bn_out[l][:])
                nc.scalar.mul(bnsc[:, 0:1], stats[:, 2:3], 1.0 / N)
                nc.scalar.mul(bnsc[:, 1:2], stats[:, 3:4], 1.0 / N)
                nc.vector.tensor_tensor(out=bnsc[:, 2:3], in0=bnsc[:, 0:1],
                                        in1=bnsc[:, 0:1], op=OP.mult)
                nc.vector.tensor_tensor(out=bnsc[:, 2:3], in0=bnsc[:, 1:2],
                                        in1=bnsc[:, 2:3], op=OP.subtract)
                nc.vector.tensor_scalar_add(bnsc[:, 2:3], bnsc[:, 2:3], BN_EPS)
                nc.vector.reciprocal(bnsc[:, 3:4], bnsc[:, 2:3])
                nc.scalar.sqrt(bnsc[:, 4:5], bnsc[:, 3:4])
                nc.vector.tensor_tensor(out=bnsc[:, 5:6], in0=bnsc[:, 4:5],
                                        in1=wsb[f"bn{l}"][:, 0:1], op=OP.mult)
                nc.vector.tensor_tensor(out=bnsc[:, 6:7], in0=bnsc[:, 0:1],
                                        in1=bnsc[:, 5:6], op=OP.mult)
                nc.vector.tensor_tensor(out=bnsc[:, 6:7], in0=wsb[f"bn{l}"][:, 1:2],
                                        in1=bnsc[:, 6:7], op=OP.subtract)
                nc.scalar.activation(hT_act[:, 0:NC_N], hT_res[:, 0:NC_N],
                                     AF.Relu, bias=bnsc[:, 6:7],
                                     scale=bnsc[:, 5:6])

            # ================= head ========================================
            out_sb = hp.tile([3, NC_N], F32, tag="osb")
            for ci in range(n_chunks):
                c0 = ci * CHUNK
                cw = min(CHUNK, NC_N - c0)
                ps_o = pp.tile([3, CHUNK], F32, tag="s", space="PSUM")
                nc.tensor.matmul(out=ps_o[:, :cw], lhsT=wsb["headW"][:],
                                 rhs=hT_act[:F, c0:c0 + cw], start=True, stop=True)
                nc.scalar.activation(out_sb[:, c0:c0 + cw], ps_o[:, :cw],
                                     AF.Identity, bias=wsb["headb"][:, 0:1])
            nc.sync.dma_start(out=out_ext[:], in_=out_sb[:, 0:NC_N])

    return nc


# ---------------------------------------------------------------- run cache
_CACHE = {}


def _build_inputs(inputs, meta, idx_full, dl_dev):
    w = pack_weights(inputs)
    x = np.asarray(inputs["x"], np.float32)
    in_maps = []
    for c in range(N_CORES):
        m = dict(w)
        m["xT"] = np.ascontiguousarray(
            x[c * NC_N:(c + 1) * NC_N, :].T).astype(BF)
        m["idx"] = np.ascontiguousarray(idx_full[c])
        m["dstloc"] = np.ascontiguousarray(dl_dev[c])
        in_maps.append(m)
    return in_maps


def kernel(**inputs):
    from concourse.bass_utils import run_bass_kernel_spmd

    _install_hookshim()
    edge = np.asarray(inputs["edge_index"])
    key = hashlib.sha1(edge.tobytes()).hexdigest()
    if key not in _CACHE:
        idx_full, dl_dev, meta = preprocess(edge)
        nc = build_program(meta)
        nc.finalize()
        if not os.environ.get("GNN_SYNC"):
            n_remap = remap_dmasw_waits(nc)
            print(f"remapped DMASW waits on {n_remap} instructions")
        n_fix = legalize_waits(nc)
        if n_fix:
            print(f"legalize_waits fixed {n_fix} instructions post-finalize")
        _CACHE[key] = (idx_full, dl_dev, meta, nc)
    idx_full, dl_dev, meta, nc = _CACHE[key]
    in_maps = _build_inputs(inputs, meta, idx_full, dl_dev)
    res = run_bass_kernel_spmd(
        nc, in_maps, list(range(N_CORES)),
        trace=bool(os.environ.get("GNN_TRACE")))
    if res.exec_time_ns is not None:
        print(f"HW exec time: {res.exec_time_ns} ns")
    out = np.concatenate([res.results[c]["out"] for c in range(N_CORES)],
                         axis=1)  # [3, N]
    return np.ascontiguousarray(out.T).astype(np.float32)


# revision 24
# speedup vs baseline: 1.1948x; 1.1948x over previous
"""AttnGraphSAGE on 8 Trainium2 NeuronCores (Bass/Tile) — v2.

Math restructuring (unchanged from v1): attention logits depend only on the
SOURCE node, so the whole edge phase is ONE segment-sum over dst of per-src
rows G[n] = [E_0*x_jm_0 (64) | E_1*x_jm_1 (64) | E_0 | E_1] (130 values).

v2 performance changes:
  * G rows are bf16, 256-elem / 512B strides (was f32 768B): halves the
    random-gather HBM traffic and the AllGather volume.  All matmul operands
    (weights, activations, indicator) are bf16 -> 1-pass PE instead of 4.
  * dma_gather calls are PREPARE_ONLY + trigger_dma: GpSimd only generates
    descriptors (~1us/call) instead of blocking until the DMA lands
    (~7us/call serialized in v1).  DMA queues stay deep and overlap compute.
  * The G table is AllGather'd in TWO halves (A = rows [0,3072) of each
    core, B = rows [3072,6250)): AG(A) overlaps phase-A compute of the B
    rows, and each half has <32768 rows so the two gather address ranges
    double as the int16-index split (no separate lo/hi split needed).
  * Exact per-block index counts (padded only to the max across the 8 cores
    so the program stays SPMD-uniform), 0-padded: no trailing -1 indices,
    ~15% fewer descriptors than v1's global-max padding.
"""
import os
import sys
import types
import hashlib
import contextlib

sys.path.insert(0, "/opt/trn_rl_repo")

import numpy as np
import ml_dtypes

import concourse.bass as bass
import concourse.bacc as bacc
import concourse.mybir as mybir
from concourse import tile

# ---------------------------------------------------------------- constants
N = 50000
E = 800000
IN = 128
F = 64
H = 2
N_CORES = 8
NC_N = N // N_CORES          # 6250 nodes per core
BLK = 128                    # dst nodes per block
ROW = 256                    # G row stride in bf16 elems (512B)
GVAL = 2 * F + H             # 130 used cols
ASPLIT = 3072                # rows per core in AllGather half A
BSPLIT = NC_N - ASPLIT       # 3178 rows in half B
CHUNK = 512                  # phase-A node chunk (6 chunks cover ASPLIT)
CAP = int(os.environ.get("GNN_CAP", "3500"))   # max idxs per gather call
F32 = mybir.dt.float32
BF16 = mybir.dt.bfloat16
I16 = mybir.dt.int16
AF = mybir.ActivationFunctionType
OP = mybir.AluOpType
BN_EPS = 1e-5
LEAKY = 0.2
BF = ml_dtypes.bfloat16


# ------------------------------------------------------- axon profile shim
def _install_hookshim():
    if "antenv.axon_hooks" in sys.modules:
        return
    mod = types.ModuleType("antenv.axon_hooks")
    _h = [None]
    mod.set_axon_ntff_profile_hook = lambda h: _h.__setitem__(0, h)
    mod.get_axon_ntff_profile_hook = lambda: _h[0]
    try:
        import antenv
        sys.modules["antenv.axon_hooks"] = mod
        antenv.axon_hooks = mod
        from trn_agent_boot.trn_boot import _ntff_profile_via_ctypes
        mod.set_axon_ntff_profile_hook(
            _ntff_profile_via_ctypes("/opt/axon/libaxon_pjrt.so")
        )
    except Exception:
        pass


def remap_dmasw_waits(nc):
    """Remap waits on Tile's DMASW lane semaphores to the per-queue gather
    DMA-completion sems.

    Tile assigned each PREPARE_ONLY gather prep a DMASW lane (round-robin)
    and derived all downstream waits (consumers, ring flow control) as
    ``DMASW{lane} >= 16*tick``.  But the sem actually baked into the
    descriptors (and bumped by the SDMA engines) is our per-queue gdma sem,
    so those lane sems never move.  Each prep records its assigned
    (lane proc, tick); since each queue's ring is FIFO, the k-th prep of
    queue q has completed exactly when gdma{q} >= 16*k.  Rewrite every
    DMASW wait for (lane, tick) into the equivalent (and race-free)
    per-queue wait."""
    from concourse.tile_sem_assignment import PROC_NAME_TO_IDX
    inv_proc = {v: k for k, v in PROC_NAME_TO_IDX.items()}

    insts = []
    for func in nc.m.functions:
        for block in func.blocks:
            insts.extend(block.instructions)

    # (lane_name, 16*tick) -> (gdma sem id, gdma name, block-level target)
    lane_map = {}
    for inst in insts:
        if type(inst).__name__ == "InstDMAGatherAnt" and \
                getattr(inst, "gen_mode", 0) == 1:
            lane = inv_proc[inst.bass_scheduled_proc]
            upd = inst.sync_info.on_update[0]
            assert upd.ant_name.startswith("gdma"), upd.ant_name
            key = (lane, 16 * inst.bass_scheduled_tick)
            assert key not in lane_map, key
            lane_map[key] = (upd.id, upd.ant_name,
                             nc._gnn_prep_targets[inst.name])

    n = 0
    n_del = 0
    for func in nc.m.functions:
        for block in func.blocks:
            kept = []
            for inst in block.instructions:
                # Tile's per-prep DMASW shadow-sem maintenance is dead weight
                # once nothing uses the lane sems (1.65us of Pool each, plus
                # serializing ring-drain waits); our one-block-per-queue gates
                # keep the ring far below capacity without it.
                if type(inst).__name__ == "InstIncSwdgeSem":
                    n_del += 1
                    continue
                kept.append(inst)
                si = inst.sync_info
                if not (si and si.on_wait):
                    continue
                changed = False
                new_waits = []
                for w in si.on_wait:
                    if w.ant_name and w.ant_name.startswith("DMASW"):
                        lane = w.ant_name.split("_")[0]
                        sid, sname, thresh = lane_map[(lane, w.wait_value)]
                        new_waits.append(mybir.SyncWait(
                            sync_type="semaphore", id=sid,
                            wait_mode="sem-ge-imm",
                            wait_value=thresh, ant_name=sname))
                        changed = True
                    else:
                        new_waits.append(w)
                if changed:
                    si.on_wait = new_waits
                    n += 1
            block.instructions[:] = kept
    return n, n_del


# ------------------------------------------------------------ wait legalize
def legalize_waits(nc):
    """TRN2 TPB instructions have ONE sync-wait slot (EventSemaphore has 2);
    hoist extra waits left by the Tile scheduler into EVSEM prequels."""
    n_fixed = 0
    for func in nc.m.functions:
        for block in func.blocks:
            new_insts = []
            for inst in block.instructions:
                si = inst.sync_info
                waits = list(si.on_wait) if si and si.on_wait else []
                cap = 2 if isinstance(inst, mybir.InstEventSemaphore) else 1
                if isinstance(inst, mybir.InstDrain):
                    cap = 1
                if len(waits) > cap:
                    extra, keep = waits[:-cap], waits[-cap:]
                    for i in range(0, len(extra), 2):
                        new_insts.append(
                            mybir.InstEventSemaphore(
                                name=nc.get_next_instruction_name(),
                                ins=[],
                                outs=[],
                                engine=inst.engine,
                                sync_info=mybir.SyncInfo(
                                    on_wait=extra[i:i + 2], on_update=[]
                                ),
                            )
                        )
                    si.on_wait = keep
                    n_fixed += 1
                new_insts.append(inst)
            block.instructions[:] = new_insts
    return n_fixed


# ----------------------------------------------------------- host preprocess
def preprocess(edge_index):
    """Sort edges by dst, partition per core / per 128-dst block, split each
    block's edges into A/B-region runs (by source row within its owner core),
    pad counts to the per-block max across cores (program is SPMD-uniform),
    and build the int16 index planes + bf16 dst-local planes."""
    nb = (NC_N + BLK - 1) // BLK
    src = np.asarray(edge_index[0], np.int64)
    dst = np.asarray(edge_index[1], np.int64)
    order = np.argsort(dst, kind="stable")
    ds, ss = dst[order], src[order]

    core = ds // NC_N
    blk = (ds - core * NC_N) // BLK
    gblk = core * nb + blk
    n_gblk = N_CORES * nb
    bbounds = np.searchsorted(gblk, np.arange(n_gblk + 1))

    # source slot within the AllGather'd table halves
    sc = ss // NC_N
    r = ss - sc * NC_N
    in_a = r < ASPLIT
    slot = np.where(in_a, sc * ASPLIT + r, sc * BSPLIT + (r - ASPLIT))

    runs = {}    # (core, block) -> (a_slots, a_dl, b_slots, b_dl)
    n_a = np.zeros((N_CORES, nb), np.int64)
    n_b = np.zeros((N_CORES, nb), np.int64)
    for g in range(n_gblk):
        e0, e1 = bbounds[g], bbounds[g + 1]
        c, b = g // nb, g % nb
        base = c * NC_N + b * BLK
        sl, dl, ia = slot[e0:e1], ds[e0:e1] - base, in_a[e0:e1]
        a_s, a_d = sl[ia], dl[ia]
        b_s, b_d = sl[~ia], dl[~ia]
        # ascending slot order inside each run -> ascending HBM addresses
        oa, ob = np.argsort(a_s, kind="stable"), np.argsort(b_s, kind="stable")
        runs[(c, b)] = (a_s[oa], a_d[oa], b_s[ob], b_d[ob])
        n_a[c, b], n_b[c, b] = len(a_s), len(b_s)

    n_a_u = n_a.max(axis=0)          # uniform (max-over-cores) counts
    n_b_u = n_b.max(axis=0)

    # per-block call layout (same for every core)
    def split_calls(n):
        if n == 0:
            return []
        k = (n + CAP - 1) // CAP
        szs = [n // k + (1 if i < n % k else 0) for i in range(k)]
        # round each call except the last up to a 128 multiple so calls
        # start on subtile boundaries of the stage tile
        out = []
        rem = n
        for i in range(k - 1):
            s = (szs[i] + 127) // 128 * 128
            out.append(s)
            rem -= s
        out.append(rem)
        return out

    blocks = []                      # per block: dict of layout info
    w_idx = 0
    tot_s = 0
    for b in range(nb):
        ca = split_calls(int(n_a_u[b]))
        cb = split_calls(int(n_b_u[b]))
        s_a = (int(n_a_u[b]) + BLK - 1) // BLK
        s_b_ = (int(n_b_u[b]) + BLK - 1) // BLK
        calls = []
        col = w_idx
        sub = 0
        for cs in ca:
            w = (cs + 15) // 16
            calls.append(("A", cs, col, sub))
            col += w
            sub += (cs + 127) // 128
        assert sub == s_a
        for cs in cb:
            w = (cs + 15) // 16
            calls.append(("B", cs, col, sub))
            col += w
            sub += (cs + 127) // 128
        assert sub == s_a + s_b_
        blocks.append(dict(n_a=int(n_a_u[b]), n_b=int(n_b_u[b]),
                           s=s_a + s_b_, s_a=s_a, calls=calls,
                           dl_off=tot_s))
        w_idx = col
        tot_s += s_a + s_b_

    idx_dev = np.zeros((N_CORES, 16, w_idx), np.int16)
    dl_dev = np.full((N_CORES, BLK, tot_s), -1.0, np.float32)

    def wrap16(vals, n_uni):
        # pad with valid dummy index 0 up to the uniform count
        a = np.zeros(((n_uni + 15) // 16 * 16,), np.int64)
        a[:len(vals)] = vals
        return a.reshape(-1, 16).T.astype(np.int16)

    for c in range(N_CORES):
        for b in range(nb):
            bl = blocks[b]
            a_s, a_d, b_s, b_d = runs[(c, b)]
            # index plane: A calls then B calls, contiguous columns
            awrap = wrap16(a_s, bl["n_a"])
            bwrap = wrap16(b_s, bl["n_b"])
            c0 = bl["calls"][0][2]
            idx_dev[c, :, c0:c0 + awrap.shape[1]] = awrap
            idx_dev[c, :, c0 + awrap.shape[1]:
                    c0 + awrap.shape[1] + bwrap.shape[1]] = bwrap
            # dst-local plane: slot k of the stage -> (p=k%128, s=k//128)
            dcol = np.full((bl["s"] * BLK,), -1.0, np.float32)
            dcol[:len(a_d)] = a_d
            dcol[bl["s_a"] * BLK:bl["s_a"] * BLK + len(b_d)] = b_d
            dl_dev[c, :, bl["dl_off"]:bl["dl_off"] + bl["s"]] = \
                dcol.reshape(bl["s"], BLK).T

    idx_full = np.tile(idx_dev, (1, 8, 1))     # replicate to 128 partitions
    s_max = max(bl["s"] for bl in blocks)
    meta = dict(nb=nb, blocks=blocks, w_idx=w_idx, tot_s=tot_s, s_max=s_max)
    return idx_full, dl_dev.astype(BF), meta


def pack_weights(inp):
    """Host-side packing of the small replicated weight tensors (bf16)."""
    def bd(av):  # [H, 2F] -> block-diag [H*F, H] halves (query, msg)
        av = np.asarray(av, np.float32)
        q = np.zeros((H * F, H), np.float32)
        m = np.zeros((H * F, H), np.float32)
        for h in range(H):
            q[h * F:(h + 1) * F, h] = av[h, :F]
            m[h * F:(h + 1) * F, h] = av[h, F:]
        return q, m

    w = {}
    for l in (0, 1):
        w[f"Wr{l}"] = np.asarray(inp[f"Wr{l}"], np.float32).astype(BF)
        w[f"Wn{l}"] = np.asarray(inp[f"Wn{l}"], np.float32).astype(BF)
        w[f"Wa{l}"] = np.asarray(inp[f"Wa{l}"], np.float32).astype(BF)
        q_, m_ = bd(inp[f"av{l}"])
        w[f"avq{l}"], w[f"avm{l}"] = q_.astype(BF), m_.astype(BF)
        w[f"bn{l}"] = np.stack(
            [np.asarray(inp[f"g{l}"], np.float32),
             np.asarray(inp[f"b{l}"], np.float32)], axis=1)  # [64,2] f32
    w["headW"] = np.asarray(inp["head_W"], np.float32).astype(BF)
    w["headb"] = np.asarray(inp["head_b"], np.float32).reshape(3, 1)
    w["iota"] = np.broadcast_to(np.arange(BLK, dtype=np.float32),
                                (BLK, BLK)).astype(BF)
    w["identb"] = np.eye(BLK, dtype=np.float32).astype(BF)
    w["identf"] = np.eye(BLK, dtype=np.float32)
    bo = np.zeros((H, H * F), np.float32)
    for h in range(H):
        bo[h, h * F:(h + 1) * F] = 1.0
    w["blkones"] = bo.astype(BF)
    return w


# ------------------------------------------------------------ device program
def build_program(meta):
    nb = meta["nb"]
    blocks = meta["blocks"]
    w_idx = meta["w_idx"]
    tot_s = meta["tot_s"]
    s_max = meta["s_max"]
    dims = [IN, F]

    nc = bacc.Bacc(None, num_swdge_queues=4)
    nc._gnn_prep_targets = {}   # prep inst name -> block-level gdma target

    # ---- I/O
    xT = nc.declare_dram_parameter("xT", [IN, NC_N], BF16, isOutput=False)
    idx_in = nc.declare_dram_parameter("idx", [BLK, w_idx], I16, isOutput=False)
    dl_in = nc.declare_dram_parameter("dstloc", [BLK, tot_s], BF16, isOutput=False)
    wext = {}
    for l in (0, 1):
        d = dims[l]
        wext[f"Wr{l}"] = nc.declare_dram_parameter(f"Wr{l}", [d, F], BF16, isOutput=False)
        wext[f"Wn{l}"] = nc.declare_dram_parameter(f"Wn{l}", [d, H * F], BF16, isOutput=False)
        wext[f"Wa{l}"] = nc.declare_dram_parameter(f"Wa{l}", [d, H * F], BF16, isOutput=False)
        wext[f"avq{l}"] = nc.declare_dram_parameter(f"avq{l}", [H * F, H], BF16, isOutput=False)
        wext[f"avm{l}"] = nc.declare_dram_parameter(f"avm{l}", [H * F, H], BF16, isOutput=False)
        wext[f"bn{l}"] = nc.declare_dram_parameter(f"bn{l}", [F, 2], F32, isOutput=False)
    wext["headW"] = nc.declare_dram_parameter("headW", [F, 3], BF16, isOutput=False)
    wext["headb"] = nc.declare_dram_parameter("headb", [3, 1], F32, isOutput=False)
    wext["iota"] = nc.declare_dram_parameter("iota", [BLK, BLK], BF16, isOutput=False)
    wext["identb"] = nc.declare_dram_parameter("identb", [BLK, BLK], BF16, isOutput=False)
    wext["identf"] = nc.declare_dram_parameter("identf", [BLK, BLK], F32, isOutput=False)
    wext["blkones"] = nc.declare_dram_parameter("blkones", [H, H * F], BF16, isOutput=False)
    out_ext = nc.declare_dram_parameter("out", [3, NC_N], F32, isOutput=True)

    # ---- internal DRAM
    g_src = [nc.dram_tensor(f"g_src{l}", [NC_N, ROW], BF16) for l in (0, 1)]
    g_fullA = [nc.dram_tensor(f"g_fullA{l}", [N_CORES * ASPLIT, ROW], BF16,
                              addr_space="Shared") for l in (0, 1)]
    g_fullB = [nc.dram_tensor(f"g_fullB{l}", [N_CORES * BSPLIT, ROW], BF16,
                              addr_space="Shared") for l in (0, 1)]
    bn_src = [nc.dram_tensor(f"bn_src{l}", [F, 2], F32) for l in (0, 1)]
    bn_out = [nc.dram_tensor(f"bn_out{l}", [F, 2], F32, addr_space="Shared")
              for l in (0, 1)]
    groups = [list(range(N_CORES))]

    n_chunks = (NC_N + CHUNK - 1) // CHUNK
    a_chunks = ASPLIT // CHUNK       # chunks covering the A half exactly
    stage_cap = int(os.environ.get("GNN_STAGE", "9"))
    layer_cap = int(os.environ.get("GNN_LAYERS", "2"))

    dma_sems = [nc.alloc_semaphore(f"gdma{q}") for q in range(4)]

    with tile.TileContext(nc) as tc:
        with contextlib.ExitStack() as ctx:
            cpool = ctx.enter_context(tc.tile_pool(name="const", bufs=1))
            wp = ctx.enter_context(tc.tile_pool(name="work", bufs=2))
            hp = ctx.enter_context(tc.tile_pool(name="resid", bufs=1))
            pp = ctx.enter_context(tc.tile_pool(name="psA", bufs=1, space="PSUM"))
            pb = ctx.enter_context(tc.tile_pool(name="psB", bufs=2, space="PSUM"))

            # ---- load constants
            wsb = {}
            for k, ext in wext.items():
                t = cpool.tile(list(ext.shape), ext.dtype, tag=k)
                nc.sync.dma_start(out=t[:], in_=ext[:])
                wsb[k] = t
            idx_sb = cpool.tile([BLK, w_idx], I16, tag="idx")
            nc.sync.dma_start(out=idx_sb[:], in_=idx_in[:])
            dl_sb = cpool.tile([BLK, tot_s], BF16, tag="dl")
            nc.sync.dma_start(out=dl_sb[:], in_=dl_in[:])

            hT_res = hp.tile([F, NC_N], F32, tag="hres")
            hT_act = hp.tile([F, NC_N], BF16, tag="hact")
            nc.vector.memset(hT_act[:], 0.0)
            scr = hp.tile([F, (NC_N + 1) // 2], F32, tag="scr")
            stats = hp.tile([F, 6], F32, tag="stats")
            bnsc = hp.tile([F, 8], F32, tag="bnsc")

            for l in (0, 1)[:layer_cap]:
                d = dims[l]
                # ================= phase A: per-node G rows + x_root =======
                for ci in range(n_chunks):
                    c0 = ci * CHUNK
                    cw = min(CHUNK, NC_N - c0)
                    if l == 0:
                        rhs = wp.tile([IN, CHUNK], BF16, tag="xchunk")
                        nc.sync.dma_start(out=rhs[:, :cw], in_=xT[:, c0:c0 + cw])
                        rhs_ap = rhs[:IN, :cw]
                    else:
                        rhs_ap = hT_act[:F, c0:c0 + cw]

                    ps_jm = pp.tile([H * F, CHUNK], F32, tag="jm", space="PSUM")
                    ps_iq = pp.tile([H * F, CHUNK], F32, tag="iq", space="PSUM")
                    ps_r = pp.tile([F, CHUNK], F32, tag="r", space="PSUM")
                    nc.tensor.matmul(out=ps_jm[:, :cw], lhsT=wsb[f"Wn{l}"][:d, :],
                                     rhs=rhs_ap, start=True, stop=True)
                    nc.tensor.matmul(out=ps_iq[:, :cw], lhsT=wsb[f"Wa{l}"][:d, :],
                                     rhs=rhs_ap, start=True, stop=True)
                    nc.tensor.matmul(out=ps_r[:, :cw], lhsT=wsb[f"Wr{l}"][:d, :],
                                     rhs=rhs_ap, start=True, stop=True)
                    nc.vector.tensor_copy(hT_res[:, c0:c0 + cw], ps_r[:, :cw])

                    jm = wp.tile([H * F, CHUNK], BF16, tag="jm_sb")
                    nc.vector.tensor_copy(jm[:, :cw], ps_jm[:, :cw])
                    # leaky(x) = max(x, 0.2x)
                    lkjm = wp.tile([H * F, CHUNK], BF16, tag="lkjm")
                    nc.scalar.mul(lkjm[:, :cw], ps_jm[:, :cw], LEAKY)
                    nc.vector.tensor_tensor(out=lkjm[:, :cw], in0=lkjm[:, :cw],
                                            in1=jm[:, :cw], op=OP.max)
                    iq = wp.tile([H * F, CHUNK], BF16, tag="iq_sb")
                    nc.vector.tensor_copy(iq[:, :cw], ps_iq[:, :cw])
                    lkiq = wp.tile([H * F, CHUNK], BF16, tag="lkiq")
                    nc.scalar.mul(lkiq[:, :cw], ps_iq[:, :cw], LEAKY)
                    nc.vector.tensor_tensor(out=lkiq[:, :cw], in0=lkiq[:, :cw],
                                            in1=iq[:, :cw], op=OP.max)
                    ps_s = pp.tile([H, CHUNK], F32, tag="s", space="PSUM")
                    nc.tensor.matmul(out=ps_s[:, :cw], lhsT=wsb[f"avq{l}"][:],
                                     rhs=lkiq[:, :cw], start=True, stop=False)
                    nc.tensor.matmul(out=ps_s[:, :cw], lhsT=wsb[f"avm{l}"][:],
                                     rhs=lkjm[:, :cw], start=False, stop=True)
                    e_sb = wp.tile([H, CHUNK], BF16, tag="esb")
                    nc.scalar.activation(e_sb[:, :cw], ps_s[:, :cw], AF.Exp)
                    # broadcast E over the per-head 64 features via matmul
                    ps_eb = pp.tile([H * F, CHUNK], F32, tag="iq", space="PSUM")
                    nc.tensor.matmul(out=ps_eb[:, :cw], lhsT=wsb["blkones"][:],
                                     rhs=e_sb[:, :cw], start=True, stop=True)
                    eb = wp.tile([H * F, CHUNK], BF16, tag="eb")
                    nc.vector.tensor_copy(eb[:, :cw], ps_eb[:, :cw])
                    y = wp.tile([H * F, CHUNK], BF16, tag="y")
                    nc.vector.tensor_tensor(out=y[:, :cw], in0=jm[:, :cw],
                                            in1=eb[:, :cw], op=OP.mult)
                    # write G rows (transpose to node-major)
                    for q in range(0, cw, BLK):
                        qw = min(BLK, cw - q)
                        ps_t = pb.tile([BLK, BLK], BF16, tag="tp", space="PSUM")
                        nc.tensor.transpose(out=ps_t[:qw, :], in_=y[:, q:q + qw],
                                            identity=wsb["identb"][:])
                        ps_e = pb.tile([BLK, BLK], BF16, tag="tp", space="PSUM")
                        nc.tensor.transpose(out=ps_e[:qw, :H], in_=e_sb[:, q:q + qw],
                                            identity=wsb["identb"][:H, :H])
                        gt = wp.tile([BLK, ROW], BF16, tag="gt")
                        nc.vector.tensor_copy(gt[:qw, 0:H * F], ps_t[:qw, :])
                        nc.vector.tensor_copy(gt[:qw, H * F:GVAL], ps_e[:qw, :H])
                        nc.sync.dma_start(
                            out=g_src[l][c0 + q:c0 + q + qw, :],
                            in_=gt[:qw, :])
                    # AllGather half A as soon as its rows are written
                    if ci == a_chunks - 1 and stage_cap >= 2:
                        nc.gpsimd.collective_compute(
                            "AllGather", OP.bypass, replica_groups=groups,
                            ins=[g_src[l][0:ASPLIT, :]], outs=[g_fullA[l][:]])

                if stage_cap < 2:
                    continue
                nc.gpsimd.collective_compute(
                    "AllGather", OP.bypass, replica_groups=groups,
                    ins=[g_src[l][ASPLIT:NC_N, :]], outs=[g_fullB[l][:]])

                # ================= phase B: gather + indicator matmul ======
                if stage_cap < 3:
                    continue
                if l == 0:
                    cum_calls = [0, 0, 0, 0]
                    prev_trigger = [None, None, None, None]
                for b in range(nb):
                    bl = blocks[b]
                    b0 = b * BLK
                    bw = min(BLK, NC_N - b0)
                    s_b = bl["s"]
                    qnum = b % 4
                    stage = wp.tile([BLK, s_max, ROW], BF16, tag="stage", bufs=3)
                    if b < 3 and l == 0:
                        nc.vector.memset(stage[:], 0.0)
                    sync_mode = bool(os.environ.get("GNN_SYNC"))
                    gate = None
                    if not sync_mode and cum_calls[qnum] > 0:
                        # drain queue q's previous block before pushing more:
                        # bounds triggered-incomplete work to ONE block per
                        # queue, which makes the cumulative 16-per-call
                        # thresholds exact (no DMA-engine skew hazard)
                        gate = nc.gpsimd.wait_ge(dma_sems[qnum],
                                                 16 * cum_calls[qnum])
                        deps = bass.InstructionNameOrderedSet()
                        deps.add(prev_trigger[qnum].ins.name)
                        gate.ins.add_nosync_dependencies_from(deps)
                    blk_preps = []
                    for reg, cs, col, sub in bl["calls"]:
                        in_view = (g_fullA[l][:] if reg == "A"
                                   else g_fullB[l][:])
                        nsub = (cs + 127) // 128
                        if sync_mode:
                            nc.gpsimd.dma_gather(
                                out_ap=stage[:, sub:sub + nsub, :],
                                in_ap=in_view,
                                idxs_ap=idx_sb[:, col:col + (cs + 15) // 16],
                                num_idxs=cs, num_idxs_reg=cs,
                                elem_size=ROW, queue_num=qnum)
                        else:
                            p = nc.gpsimd.dma_gather(
                                out_ap=stage[:, sub:sub + nsub, :],
                                in_ap=in_view,
                                idxs_ap=idx_sb[:, col:col + (cs + 15) // 16],
                                num_idxs=cs, num_idxs_reg=cs,
                                elem_size=ROW, queue_num=qnum,
                                prepare_only=True, sem=dma_sems[qnum])
                            cum_calls[qnum] += 1
                            blk_preps.append(p)
                            # keep per-queue ring order == program order:
                            # a prep may not be scheduled before the previous
                            # trigger of its queue (else that trigger fires
                            # this prep's descriptors)
                            deps = bass.InstructionNameOrderedSet()
                            have = False
                            if gate is not None:
                                deps.add(gate.ins.name)
                                have = True
                            if prev_trigger[qnum] is not None:
                                deps.add(prev_trigger[qnum].ins.name)
                                have = True
                            if have:
                                p.ins.add_nosync_dependencies_from(deps)
                    if not sync_mode:
                        prev_trigger[qnum] = nc.gpsimd.trigger_dma(
                            count=None, queue_num=qnum)
                        # consumers must wait for the whole block's calls
                        for p in blk_preps:
                            nc._gnn_prep_targets[p.ins.name] = \
                                16 * cum_calls[qnum]
                    if stage_cap < 4:
                        continue
                    off = bl["dl_off"]
                    ind = wp.tile([BLK, s_max * BLK], BF16, tag="ind", bufs=3)
                    nc.vector.tensor_tensor(
                        out=ind[:, 0:s_b * BLK].rearrange("p (s i) -> p s i", i=BLK),
                        in0=dl_sb[:, off:off + s_b][:, :, None]
                            .to_broadcast([BLK, s_b, BLK]),
                        in1=wsb["iota"][:, None, :].to_broadcast([BLK, s_b, BLK]),
                        op=OP.is_equal)
                    ps_blk = pb.tile([BLK, GVAL], F32, tag="blk", space="PSUM")
                    for j in range(s_b):
                        mm = nc.tensor.matmul(out=ps_blk[:],
                                              lhsT=ind[:, j * BLK:(j + 1) * BLK],
                                              rhs=stage[:, j, 0:GVAL],
                                              start=(j == 0), stop=(j == s_b - 1))
                        if not sync_mode and mm is not None:
                            # scheduling-order (no-sync) edge: keep the
                            # stage consumers after this block's trigger in
                            # the PE stream, else PE head-of-line blocks on
                            # data whose trigger hasn't dispatched yet
                            deps = bass.InstructionNameOrderedSet()
                            deps.add(prev_trigger[qnum].ins.name)
                            mm.ins.add_nosync_dependencies_from(deps)
                    sb = wp.tile([BLK, GVAL], F32, tag="sbblk")
                    nc.vector.tensor_copy(sb[:], ps_blk[:])
                    rec = wp.tile([BLK, H], F32, tag="rec")
                    nc.vector.tensor_scalar_add(rec[:], sb[:, H * F:GVAL], 1e-30)
                    nc.vector.reciprocal(rec[:], rec[:])
                    agg = wp.tile([BLK, F], F32, tag="agg")
                    tmp = wp.tile([BLK, F], F32, tag="tmp")
                    nc.vector.tensor_tensor(out=agg[:], in0=sb[:, 0:F],
                                            in1=rec[:, 0:1].to_broadcast([BLK, F]),
                                            op=OP.mult)
                    nc.vector.tensor_tensor(out=tmp[:], in0=sb[:, F:2 * F],
                                            in1=rec[:, 1:2].to_broadcast([BLK, F]),
                                            op=OP.mult)
                    nc.vector.tensor_add(out=agg[:], in0=agg[:], in1=tmp[:])
                    agg_bf = wp.tile([BLK, F], BF16, tag="aggbf")
                    nc.vector.tensor_copy(agg_bf[:], agg[:])
                    ps_t = pb.tile([BLK, BLK], BF16, tag="tp", space="PSUM")
                    nc.tensor.transpose(out=ps_t[:F, :], in_=agg_bf[:, :F],
                                        identity=wsb["identb"][:])
                    nc.vector.tensor_add(out=hT_res[:, b0:b0 + bw],
                                         in0=hT_res[:, b0:b0 + bw],
                                         in1=ps_t[:F, :bw])

                # ================= BatchNorm + ReLU ========================
                if stage_cap < 5:
                    continue
                nc.vector.reduce_sum(out=stats[:, 0:1], in_=hT_res[:, 0:NC_N],
                                     axis=mybir.AxisListType.X)
                half = (NC_N + 1) // 2
                nc.scalar.square(scr[:, 0:half], hT_res[:, 0:half])
                nc.vector.reduce_sum(out=stats[:, 1:2], in_=scr[:, 0:half],
                                     axis=mybir.AxisListType.X)
                nc.scalar.square(scr[:, 0:NC_N - half], hT_res[:, half:NC_N])
                nc.vector.reduce_sum(out=stats[:, 4:5], in_=scr[:, 0:NC_N - half],
                                     axis=mybir.AxisListType.X)
                nc.vector.tensor_add(out=stats[:, 1:2], in0=stats[:, 1:2],
                                     in1=stats[:, 4:5])
                nc.sync.dma_start(out=bn_src[l][:], in_=stats[:, 0:2])
                nc.gpsimd.collective_compute(
                    "AllReduce", OP.add, replica_groups=groups,
                    ins=[bn_src[l][:]], outs=[bn_out[l][:]])
                nc.sync.dma_start(out=stats[:, 2:4], in_=bn_out[l][:])
                nc.scalar.mul(bnsc[:, 0:1], stats[:, 2:3], 1.0 / N)
                nc.scalar.mul(bnsc[:, 1:2], stats[:, 3:4], 1.0 / N)
                nc.vector.tensor_tensor(out=bnsc[:, 2:3], in0=bnsc[:, 0:1],
                                        in1=bnsc[:, 0:1], op=OP.mult)
                nc.vector.tensor_tensor(out=bnsc[:, 2:3], in0=bnsc[:, 1:2],
                                        in1=bnsc[:, 2:3], op=OP.subtract)
                nc.vector.tensor_scalar_add(bnsc[:, 2:3], bnsc[:, 2:3], BN_EPS)
                nc.vector.reciprocal(bnsc[:, 3:4], bnsc[:, 2:3])
                nc.scalar.sqrt(bnsc[:, 4:5], bnsc[:, 3:4])
                nc.vector.tensor_tensor(out=bnsc[:, 5:6], in0=bnsc[:, 4:5],
                                        in1=wsb[f"bn{l}"][:, 0:1], op=OP.mult)
                nc.vector.tensor_tensor(out=bnsc[:, 6:7], in0=bnsc[:, 0:1],
                                        in1=bnsc[:, 5:6], op=OP.mult)
                nc.vector.tensor_tensor(out=bnsc[:, 6:7], in0=wsb[f"bn{l}"][:, 1:2],
                                        in1=bnsc[:, 6:7], op=OP.subtract)
                nc.scalar.activation(hT_act[:, 0:NC_N], hT_res[:, 0:NC_N],
                                     AF.Relu, bias=bnsc[:, 6:7],
                                     scale=bnsc[:, 5:6])

            # ================= head ========================================
            out_sb = hp.tile([3, NC_N], F32, tag="osb")
            for ci in range(n_chunks):
                c0 = ci * CHUNK
                cw = min(CHUNK, NC_N - c0)
                ps_o = pp.tile([3, CHUNK], F32, tag="s", space="PSUM")
                nc.tensor.matmul(out=ps_o[:, :cw], lhsT=wsb["headW"][:],
                                 rhs=hT_act[:F, c0:c0 + cw], start=True, stop=True)
                nc.scalar.activation(out_sb[:, c0:c0 + cw], ps_o[:, :cw],
                                     AF.Identity, bias=wsb["headb"][:, 0:1])
            nc.sync.dma_start(out=out_ext[:], in_=out_sb[:, 0:NC_N])

    return nc


# ---------------------------------------------------------------- run cache
_CACHE = {}


def _build_inputs(inputs, meta, idx_full, dl_dev):
    w = pack_weights(inputs)
    x = np.asarray(inputs["x"], np.float32)
    in_maps = []
    for c in range(N_CORES):
        m = dict(w)
        m["xT"] = np.ascontiguousarray(
            x[c * NC_N:(c + 1) * NC_N, :].T).astype(BF)
        m["idx"] = np.ascontiguousarray(idx_full[c])
        m["dstloc"] = np.ascontiguousarray(dl_dev[c])
        in_maps.append(m)
    return in_maps


def kernel(**inputs):
    from concourse.bass_utils import run_bass_kernel_spmd

    _install_hookshim()
    edge = np.asarray(inputs["edge_index"])
    key = hashlib.sha1(edge.tobytes()).hexdigest()
    if key not in _CACHE:
        idx_full, dl_dev, meta = preprocess(edge)
        nc = build_program(meta)
        nc.finalize()
        if not os.environ.get("GNN_SYNC"):
            n_remap, n_del = remap_dmasw_waits(nc)
            print(f"remapped DMASW waits on {n_remap} instructions, "
                  f"deleted {n_del} IncSwdgeSem")
        n_fix = legalize_waits(nc)
        if n_fix:
            print(f"legalize_waits fixed {n_fix} instructions post-finalize")
        _CACHE[key] = (idx_full, dl_dev, meta, nc)
    idx_full, dl_dev, meta, nc = _CACHE[key]
    in_maps = _build_inputs(inputs, meta, idx_full, dl_dev)
    res = run_bass_kernel_spmd(
        nc, in_maps, list(range(N_CORES)),
        trace=bool(os.environ.get("GNN_TRACE")))
    if res.exec_time_ns is not None:
        print(f"HW exec time: {res.exec_time_ns} ns")
    out = np.concatenate([res.results[c]["out"] for c in range(N_CORES)],
                         axis=1)  # [3, N]
    return np.ascontiguousarray(out.T).astype(np.float32)


# revision 28
# speedup vs baseline: 1.2053x; 1.0088x over previous
"""AttnGraphSAGE on 8 Trainium2 NeuronCores (Bass/Tile) — v2.

Math restructuring (unchanged from v1): attention logits depend only on the
SOURCE node, so the whole edge phase is ONE segment-sum over dst of per-src
rows G[n] = [E_0*x_jm_0 (64) | E_1*x_jm_1 (64) | E_0 | E_1] (130 values).

v2 performance changes:
  * G rows are bf16, 256-elem / 512B strides (was f32 768B): halves the
    random-gather HBM traffic and the AllGather volume.  All matmul operands
    (weights, activations, indicator) are bf16 -> 1-pass PE instead of 4.
  * dma_gather calls are PREPARE_ONLY + trigger_dma: GpSimd only generates
    descriptors (~1us/call) instead of blocking until the DMA lands
    (~7us/call serialized in v1).  DMA queues stay deep and overlap compute.
  * The G table is AllGather'd in TWO halves (A = rows [0,3072) of each
    core, B = rows [3072,6250)): AG(A) overlaps phase-A compute of the B
    rows, and each half has <32768 rows so the two gather address ranges
    double as the int16-index split (no separate lo/hi split needed).
  * Exact per-block index counts (padded only to the max across the 8 cores
    so the program stays SPMD-uniform), 0-padded: no trailing -1 indices,
    ~15% fewer descriptors than v1's global-max padding.
"""
import os
import sys
import types
import hashlib
import contextlib

sys.path.insert(0, "/opt/trn_rl_repo")

import numpy as np
import ml_dtypes

import concourse.bass as bass
import concourse.bacc as bacc
import concourse.mybir as mybir
from concourse import tile

# ---------------------------------------------------------------- constants
N = 50000
E = 800000
IN = 128
F = 64
H = 2
N_CORES = 8
NC_N = N // N_CORES          # 6250 nodes per core
BLK = 128                    # dst nodes per block
ROW = 256                    # G row stride in bf16 elems (512B)
GVAL = 2 * F + H             # 130 used cols
ASPLIT = 3072                # rows per core in AllGather half A
BSPLIT = NC_N - ASPLIT       # 3178 rows in half B
CHUNK = 512                  # phase-A node chunk (6 chunks cover ASPLIT)
CAP = int(os.environ.get("GNN_CAP", "1024"))   # idxs per gather call (HW max)
ARENA_S = 32                 # ring-arena subtiles per region (4 calls)
F32 = mybir.dt.float32
BF16 = mybir.dt.bfloat16
I16 = mybir.dt.int16
AF = mybir.ActivationFunctionType
OP = mybir.AluOpType
BN_EPS = 1e-5
LEAKY = 0.2
BF = ml_dtypes.bfloat16


# ------------------------------------------------------- axon profile shim
def _install_hookshim():
    if "antenv.axon_hooks" in sys.modules:
        return
    mod = types.ModuleType("antenv.axon_hooks")
    _h = [None]
    mod.set_axon_ntff_profile_hook = lambda h: _h.__setitem__(0, h)
    mod.get_axon_ntff_profile_hook = lambda: _h[0]
    try:
        import antenv
        sys.modules["antenv.axon_hooks"] = mod
        antenv.axon_hooks = mod
        from trn_agent_boot.trn_boot import _ntff_profile_via_ctypes
        mod.set_axon_ntff_profile_hook(
            _ntff_profile_via_ctypes("/opt/axon/libaxon_pjrt.so")
        )
    except Exception:
        pass


def remap_dmasw_waits(nc):
    """Remap waits on Tile's DMASW lane semaphores to the per-queue gather
    DMA-completion sems.

    Tile assigned each PREPARE_ONLY gather prep a DMASW lane (round-robin)
    and derived all downstream waits (consumers, ring flow control) as
    ``DMASW{lane} >= 16*tick``.  But the sem actually baked into the
    descriptors (and bumped by the SDMA engines) is our per-queue gdma sem,
    so those lane sems never move.  Each prep records its assigned
    (lane proc, tick); since each queue's ring is FIFO, the k-th prep of
    queue q has completed exactly when gdma{q} >= 16*k.  Rewrite every
    DMASW wait for (lane, tick) into the equivalent (and race-free)
    per-queue wait."""
    from concourse.tile_sem_assignment import PROC_NAME_TO_IDX
    inv_proc = {v: k for k, v in PROC_NAME_TO_IDX.items()}

    insts = []
    for func in nc.m.functions:
        for block in func.blocks:
            insts.extend(block.instructions)

    # (lane_name, 16*tick) -> (gdma sem id, gdma name, block-level target)
    lane_map = {}
    for inst in insts:
        if type(inst).__name__ == "InstDMAGatherAnt" and \
                getattr(inst, "gen_mode", 0) == 1:
            lane = inv_proc[inst.bass_scheduled_proc]
            upd = inst.sync_info.on_update[0]
            assert upd.ant_name.startswith("gdma"), upd.ant_name
            key = (lane, 16 * inst.bass_scheduled_tick)
            assert key not in lane_map, key
            lane_map[key] = (upd.id, upd.ant_name,
                             nc._gnn_prep_targets[inst.name])

    n = 0
    n_del = 0
    for func in nc.m.functions:
        for block in func.blocks:
            kept = []
            for inst in block.instructions:
                # Tile's per-prep DMASW shadow-sem maintenance is dead weight
                # once nothing uses the lane sems (1.65us of Pool each, plus
                # serializing ring-drain waits); our one-block-per-queue gates
                # keep the ring far below capacity without it.
                if type(inst).__name__ == "InstIncSwdgeSem":
                    n_del += 1
                    continue
                kept.append(inst)
                si = inst.sync_info
                if not (si and si.on_wait):
                    continue
                changed = False
                new_waits = []
                for w in si.on_wait:
                    if w.ant_name and w.ant_name.startswith("DMASW"):
                        lane = w.ant_name.split("_")[0]
                        sid, sname, thresh = lane_map[(lane, w.wait_value)]
                        new_waits.append(mybir.SyncWait(
                            sync_type="semaphore", id=sid,
                            wait_mode="sem-ge-imm",
                            wait_value=thresh, ant_name=sname))
                        changed = True
                    else:
                        new_waits.append(w)
                if changed:
                    si.on_wait = new_waits
                    n += 1
            block.instructions[:] = kept
    return n, n_del


# ------------------------------------------------------------ wait legalize
def legalize_waits(nc):
    """TRN2 TPB instructions have ONE sync-wait slot (EventSemaphore has 2);
    hoist extra waits left by the Tile scheduler into EVSEM prequels."""
    n_fixed = 0
    for func in nc.m.functions:
        for block in func.blocks:
            new_insts = []
            for inst in block.instructions:
                si = inst.sync_info
                waits = list(si.on_wait) if si and si.on_wait else []
                cap = 2 if isinstance(inst, mybir.InstEventSemaphore) else 1
                if isinstance(inst, mybir.InstDrain):
                    cap = 1
                if len(waits) > cap:
                    extra, keep = waits[:-cap], waits[-cap:]
                    for i in range(0, len(extra), 2):
                        new_insts.append(
                            mybir.InstEventSemaphore(
                                name=nc.get_next_instruction_name(),
                                ins=[],
                                outs=[],
                                engine=inst.engine,
                                sync_info=mybir.SyncInfo(
                                    on_wait=extra[i:i + 2], on_update=[]
                                ),
                            )
                        )
                    si.on_wait = keep
                    n_fixed += 1
                new_insts.append(inst)
            block.instructions[:] = new_insts
    return n_fixed


# ----------------------------------------------------------- host preprocess
def preprocess(edge_index):
    """Sort edges by dst, partition per core / per 128-dst block, split each
    block's edges into A/B-region runs (by source row within its owner core),
    pad counts to the per-block max across cores (program is SPMD-uniform).

    Each region's padded edge stream is then PACKED into gather calls of
    exactly CAP indices spanning block boundaries (the Q7 per-call fixed
    cost ~4us dominates, so call count is what matters).  Calls write 8
    consecutive subtiles of a 32-subtile ring arena per region; a block's
    indicator matmul consumes the (possibly boundary-shared) subtiles it
    touches, with foreign slots killed by dl=-1."""
    nb = (NC_N + BLK - 1) // BLK
    src = np.asarray(edge_index[0], np.int64)
    dst = np.asarray(edge_index[1], np.int64)
    order = np.argsort(dst, kind="stable")
    ds, ss = dst[order], src[order]

    core = ds // NC_N
    blk = (ds - core * NC_N) // BLK
    gblk = core * nb + blk
    n_gblk = N_CORES * nb
    bbounds = np.searchsorted(gblk, np.arange(n_gblk + 1))

    # source slot within the AllGather'd table halves
    sc = ss // NC_N
    r = ss - sc * NC_N
    in_a = r < ASPLIT
    slot = np.where(in_a, sc * ASPLIT + r, sc * BSPLIT + (r - ASPLIT))

    runs = {}    # (core, block) -> (a_slots, a_dl, b_slots, b_dl)
    n_a = np.zeros((N_CORES, nb), np.int64)
    n_b = np.zeros((N_CORES, nb), np.int64)
    for g in range(n_gblk):
        e0, e1 = bbounds[g], bbounds[g + 1]
        c, b = g // nb, g % nb
        base = c * NC_N + b * BLK
        sl, dl, ia = slot[e0:e1], ds[e0:e1] - base, in_a[e0:e1]
        a_s, a_d = sl[ia], dl[ia]
        b_s, b_d = sl[~ia], dl[~ia]
        # ascending slot order inside each run -> ascending HBM addresses
        oa, ob = np.argsort(a_s, kind="stable"), np.argsort(b_s, kind="stable")
        runs[(c, b)] = (a_s[oa], a_d[oa], b_s[ob], b_d[ob])
        n_a[c, b], n_b[c, b] = len(a_s), len(b_s)

    n_a_u = n_a.max(axis=0).astype(int)   # uniform (max-over-cores) counts
    n_b_u = n_b.max(axis=0).astype(int)

    # region stream layout: block b's run occupies [start[b], start[b]+n)
    def region_layout(n_u):
        starts = np.concatenate([[0], np.cumsum(n_u)])
        total = int(starts[-1])
        total_pad = (total + CAP - 1) // CAP * CAP   # pad last call
        n_calls = total_pad // CAP
        n_sub = total_pad // BLK
        return starts, total, total_pad, n_calls, n_sub

    sa_starts, sa_tot, sa_pad, na_calls, _ = region_layout(n_a_u)
    sb_starts, sb_tot, sb_pad, nb_calls, _ = region_layout(n_b_u)

    # per block: touched subtiles per region + dl columns
    blocks = []
    tot_s = 0
    for b in range(nb):
        entry = dict(dl_off=tot_s, subs=[])   # subs: (region, glob_subtile)
        for reg, starts, n_u in (("A", sa_starts, n_a_u), ("B", sb_starts, n_b_u)):
            e0, e1 = int(starts[b]), int(starts[b] + n_u[b])
            s0, s1 = e0 // BLK, (e1 + BLK - 1) // BLK
            for s in range(s0, s1):
                entry["subs"].append((reg, s, e0, e1))
            # calls needed (exclusive prefix): region call idx covering e1-1
            entry[f"need{reg}"] = (e1 + CAP - 1) // CAP if e1 > 0 else 0
        entry["n_sub"] = len(entry["subs"])
        tot_s += entry["n_sub"]
        blocks.append(entry)

    # index planes: region streams wrapped per call (CAP idx = CAP//16 cols)
    wA, wB = na_calls * (CAP // 16), nb_calls * (CAP // 16)
    w_idx = wA + wB
    idx_dev = np.zeros((N_CORES, 16, w_idx), np.int16)
    dl_dev = np.full((N_CORES, BLK, tot_s), -1.0, np.float32)

    for c in range(N_CORES):
        for reg, starts, n_u, pad_tot, col0 in (
                ("A", sa_starts, n_a_u, sa_pad, 0),
                ("B", sb_starts, n_b_u, sb_pad, wA)):
            streamv = np.zeros((pad_tot,), np.int64)
            for b in range(nb):
                a_s, a_d, b_s, b_d = runs[(c, b)]
                v = a_s if reg == "A" else b_s
                e0 = int(starts[b])
                streamv[e0:e0 + len(v)] = v
            # wrap16 whole region stream: idx i -> (p=i%16, col=i//16)
            idx_dev[c, :, col0:col0 + pad_tot // 16] = \
                streamv.reshape(-1, 16).T.astype(np.int16)
        # dl columns
        for b in range(nb):
            bl = blocks[b]
            a_s, a_d, b_s, b_d = runs[(c, b)]
            for k, (reg, s, e0, e1) in enumerate(bl["subs"]):
                dvals = a_d if reg == "A" else b_d
                base = e0
                col = np.full((BLK,), -1.0, np.float32)
                lo = max(e0, s * BLK)
                hi = min(e0 + len(dvals), (s + 1) * BLK)
                if hi > lo:
                    col[lo - s * BLK:hi - s * BLK] = dvals[lo - base:hi - base]
                dl_dev[c, :, bl["dl_off"] + k] = col

    idx_full = np.tile(idx_dev, (1, 8, 1))     # replicate to 128 partitions
    s_max = max(bl["n_sub"] for bl in blocks)
    meta = dict(nb=nb, blocks=blocks, w_idx=w_idx, tot_s=tot_s, s_max=s_max,
                na_calls=na_calls, nb_calls=nb_calls,
                sa_pad=sa_pad, sb_pad=sb_pad, wA=wA)
    return idx_full, dl_dev.astype(BF), meta


def pack_weights(inp):
    """Host-side packing of the small replicated weight tensors (bf16)."""
    def bd(av):  # [H, 2F] -> block-diag [H*F, H] halves (query, msg)
        av = np.asarray(av, np.float32)
        q = np.zeros((H * F, H), np.float32)
        m = np.zeros((H * F, H), np.float32)
        for h in range(H):
            q[h * F:(h + 1) * F, h] = av[h, :F]
            m[h * F:(h + 1) * F, h] = av[h, F:]
        return q, m

    w = {}
    for l in (0, 1):
        w[f"Wr{l}"] = np.asarray(inp[f"Wr{l}"], np.float32).astype(BF)
        w[f"Wn{l}"] = np.asarray(inp[f"Wn{l}"], np.float32).astype(BF)
        w[f"Wa{l}"] = np.asarray(inp[f"Wa{l}"], np.float32).astype(BF)
        q_, m_ = bd(inp[f"av{l}"])
        w[f"avq{l}"], w[f"avm{l}"] = q_.astype(BF), m_.astype(BF)
        w[f"bn{l}"] = np.stack(
            [np.asarray(inp[f"g{l}"], np.float32),
             np.asarray(inp[f"b{l}"], np.float32)], axis=1)  # [64,2] f32
    w["headW"] = np.asarray(inp["head_W"], np.float32).astype(BF)
    w["headb"] = np.asarray(inp["head_b"], np.float32).reshape(3, 1)
    w["iota"] = np.broadcast_to(np.arange(BLK, dtype=np.float32),
                                (BLK, BLK)).astype(BF)
    w["identb"] = np.eye(BLK, dtype=np.float32).astype(BF)
    w["identf"] = np.eye(BLK, dtype=np.float32)
    bo = np.zeros((H, H * F), np.float32)
    for h in range(H):
        bo[h, h * F:(h + 1) * F] = 1.0
    w["blkones"] = bo.astype(BF)
    return w


# ------------------------------------------------------------ device program
def build_program(meta):
    nb = meta["nb"]
    blocks = meta["blocks"]
    w_idx = meta["w_idx"]
    tot_s = meta["tot_s"]
    s_max = meta["s_max"]
    dims = [IN, F]

    nc = bacc.Bacc(None, num_swdge_queues=4)
    nc._gnn_prep_targets = {}   # prep inst name -> block-level gdma target

    # ---- I/O
    xT = nc.declare_dram_parameter("xT", [IN, NC_N], BF16, isOutput=False)
    idx_in = nc.declare_dram_parameter("idx", [BLK, w_idx], I16, isOutput=False)
    dl_in = nc.declare_dram_parameter("dstloc", [BLK, tot_s], BF16, isOutput=False)
    wext = {}
    for l in (0, 1):
        d = dims[l]
        wext[f"Wr{l}"] = nc.declare_dram_parameter(f"Wr{l}", [d, F], BF16, isOutput=False)
        wext[f"Wn{l}"] = nc.declare_dram_parameter(f"Wn{l}", [d, H * F], BF16, isOutput=False)
        wext[f"Wa{l}"] = nc.declare_dram_parameter(f"Wa{l}", [d, H * F], BF16, isOutput=False)
        wext[f"avq{l}"] = nc.declare_dram_parameter(f"avq{l}", [H * F, H], BF16, isOutput=False)
        wext[f"avm{l}"] = nc.declare_dram_parameter(f"avm{l}", [H * F, H], BF16, isOutput=False)
        wext[f"bn{l}"] = nc.declare_dram_parameter(f"bn{l}", [F, 2], F32, isOutput=False)
    wext["headW"] = nc.declare_dram_parameter("headW", [F, 3], BF16, isOutput=False)
    wext["headb"] = nc.declare_dram_parameter("headb", [3, 1], F32, isOutput=False)
    wext["iota"] = nc.declare_dram_parameter("iota", [BLK, BLK], BF16, isOutput=False)
    wext["identb"] = nc.declare_dram_parameter("identb", [BLK, BLK], BF16, isOutput=False)
    wext["identf"] = nc.declare_dram_parameter("identf", [BLK, BLK], F32, isOutput=False)
    wext["blkones"] = nc.declare_dram_parameter("blkones", [H, H * F], BF16, isOutput=False)
    out_ext = nc.declare_dram_parameter("out", [3, NC_N], F32, isOutput=True)

    # ---- internal DRAM
    g_src = [nc.dram_tensor(f"g_src{l}", [NC_N, ROW], BF16) for l in (0, 1)]
    g_fullA = [nc.dram_tensor(f"g_fullA{l}", [N_CORES * ASPLIT, ROW], BF16,
                              addr_space="Shared") for l in (0, 1)]
    g_fullB = [nc.dram_tensor(f"g_fullB{l}", [N_CORES * BSPLIT, ROW], BF16,
                              addr_space="Shared") for l in (0, 1)]
    bn_src = [nc.dram_tensor(f"bn_src{l}", [F, 2], F32) for l in (0, 1)]
    bn_out = [nc.dram_tensor(f"bn_out{l}", [F, 2], F32, addr_space="Shared")
              for l in (0, 1)]
    groups = [list(range(N_CORES))]

    n_chunks = (NC_N + CHUNK - 1) // CHUNK
    a_chunks = ASPLIT // CHUNK       # chunks covering the A half exactly
    stage_cap = int(os.environ.get("GNN_STAGE", "9"))
    layer_cap = int(os.environ.get("GNN_LAYERS", "2"))

    dma_sems = [nc.alloc_semaphore(f"gdma{q}") for q in range(4)]

    with tile.TileContext(nc) as tc:
        with contextlib.ExitStack() as ctx:
            cpool = ctx.enter_context(tc.tile_pool(name="const", bufs=1))
            wp = ctx.enter_context(tc.tile_pool(name="work", bufs=2))
            hp = ctx.enter_context(tc.tile_pool(name="resid", bufs=1))
            pp = ctx.enter_context(tc.tile_pool(name="psA", bufs=1, space="PSUM"))
            pb = ctx.enter_context(tc.tile_pool(name="psB", bufs=2, space="PSUM"))

            # ---- load constants
            wsb = {}
            for k, ext in wext.items():
                t = cpool.tile(list(ext.shape), ext.dtype, tag=k)
                nc.sync.dma_start(out=t[:], in_=ext[:])
                wsb[k] = t
            idx_sb = cpool.tile([BLK, w_idx], I16, tag="idx")
            nc.sync.dma_start(out=idx_sb[:], in_=idx_in[:])
            dl_sb = cpool.tile([BLK, tot_s], BF16, tag="dl")
            nc.sync.dma_start(out=dl_sb[:], in_=dl_in[:])

            hT_res = hp.tile([F, NC_N], F32, tag="hres")
            hT_act = hp.tile([F, NC_N], BF16, tag="hact")
            nc.vector.memset(hT_act[:], 0.0)
            arenaA = hp.tile([BLK, ARENA_S, ROW], BF16, tag="arA")
            arenaB = hp.tile([BLK, ARENA_S, ROW], BF16, tag="arB")
            scr = hp.tile([F, (NC_N + 1) // 2], F32, tag="scr")
            stats = hp.tile([F, 6], F32, tag="stats")
            bnsc = hp.tile([F, 8], F32, tag="bnsc")

            for l in (0, 1)[:layer_cap]:
                d = dims[l]
                # ================= phase A: per-node G rows + x_root =======
                for ci in range(n_chunks):
                    c0 = ci * CHUNK
                    cw = min(CHUNK, NC_N - c0)
                    if l == 0:
                        rhs = wp.tile([IN, CHUNK], BF16, tag="xchunk")
                        nc.sync.dma_start(out=rhs[:, :cw], in_=xT[:, c0:c0 + cw])
                        rhs_ap = rhs[:IN, :cw]
                    else:
                        rhs_ap = hT_act[:F, c0:c0 + cw]

                    ps_jm = pp.tile([H * F, CHUNK], F32, tag="jm", space="PSUM")
                    ps_iq = pp.tile([H * F, CHUNK], F32, tag="iq", space="PSUM")
                    ps_r = pp.tile([F, CHUNK], F32, tag="r", space="PSUM")
                    nc.tensor.matmul(out=ps_jm[:, :cw], lhsT=wsb[f"Wn{l}"][:d, :],
                                     rhs=rhs_ap, start=True, stop=True)
                    nc.tensor.matmul(out=ps_iq[:, :cw], lhsT=wsb[f"Wa{l}"][:d, :],
                                     rhs=rhs_ap, start=True, stop=True)
                    nc.tensor.matmul(out=ps_r[:, :cw], lhsT=wsb[f"Wr{l}"][:d, :],
                                     rhs=rhs_ap, start=True, stop=True)
                    nc.vector.tensor_copy(hT_res[:, c0:c0 + cw], ps_r[:, :cw])

                    jm = wp.tile([H * F, CHUNK], BF16, tag="jm_sb")
                    nc.vector.tensor_copy(jm[:, :cw], ps_jm[:, :cw])
                    # leaky(x) = max(x, 0.2x)
                    lkjm = wp.tile([H * F, CHUNK], BF16, tag="lkjm")
                    nc.scalar.mul(lkjm[:, :cw], ps_jm[:, :cw], LEAKY)
                    nc.vector.tensor_tensor(out=lkjm[:, :cw], in0=lkjm[:, :cw],
                                            in1=jm[:, :cw], op=OP.max)
                    iq = wp.tile([H * F, CHUNK], BF16, tag="iq_sb")
                    nc.vector.tensor_copy(iq[:, :cw], ps_iq[:, :cw])
                    lkiq = wp.tile([H * F, CHUNK], BF16, tag="lkiq")
                    nc.scalar.mul(lkiq[:, :cw], ps_iq[:, :cw], LEAKY)
                    nc.vector.tensor_tensor(out=lkiq[:, :cw], in0=lkiq[:, :cw],
                                            in1=iq[:, :cw], op=OP.max)
                    ps_s = pp.tile([H, CHUNK], F32, tag="s", space="PSUM")
                    nc.tensor.matmul(out=ps_s[:, :cw], lhsT=wsb[f"avq{l}"][:],
                                     rhs=lkiq[:, :cw], start=True, stop=False)
                    nc.tensor.matmul(out=ps_s[:, :cw], lhsT=wsb[f"avm{l}"][:],
                                     rhs=lkjm[:, :cw], start=False, stop=True)
                    e_sb = wp.tile([H, CHUNK], BF16, tag="esb")
                    nc.scalar.activation(e_sb[:, :cw], ps_s[:, :cw], AF.Exp)
                    # broadcast E over the per-head 64 features via matmul
                    ps_eb = pp.tile([H * F, CHUNK], F32, tag="iq", space="PSUM")
                    nc.tensor.matmul(out=ps_eb[:, :cw], lhsT=wsb["blkones"][:],
                                     rhs=e_sb[:, :cw], start=True, stop=True)
                    eb = wp.tile([H * F, CHUNK], BF16, tag="eb")
                    nc.vector.tensor_copy(eb[:, :cw], ps_eb[:, :cw])
                    y = wp.tile([H * F, CHUNK], BF16, tag="y")
                    nc.vector.tensor_tensor(out=y[:, :cw], in0=jm[:, :cw],
                                            in1=eb[:, :cw], op=OP.mult)
                    # write G rows (transpose to node-major)
                    for q in range(0, cw, BLK):
                        qw = min(BLK, cw - q)
                        ps_t = pb.tile([BLK, BLK], BF16, tag="tp", space="PSUM")
                        nc.tensor.transpose(out=ps_t[:qw, :], in_=y[:, q:q + qw],
                                            identity=wsb["identb"][:])
                        ps_e = pb.tile([BLK, BLK], BF16, tag="tp", space="PSUM")
                        nc.tensor.transpose(out=ps_e[:qw, :H], in_=e_sb[:, q:q + qw],
                                            identity=wsb["identb"][:H, :H])
                        gt = wp.tile([BLK, ROW], BF16, tag="gt")
                        nc.vector.tensor_copy(gt[:qw, 0:H * F], ps_t[:qw, :])
                        nc.vector.tensor_copy(gt[:qw, H * F:GVAL], ps_e[:qw, :H])
                        nc.sync.dma_start(
                            out=g_src[l][c0 + q:c0 + q + qw, :],
                            in_=gt[:qw, :])
                    # AllGather half A as soon as its rows are written
                    if ci == a_chunks - 1 and stage_cap >= 2:
                        nc.gpsimd.collective_compute(
                            "AllGather", OP.bypass, replica_groups=groups,
                            ins=[g_src[l][0:ASPLIT, :]], outs=[g_fullA[l][:]])

                if stage_cap < 2:
                    continue
                nc.gpsimd.collective_compute(
                    "AllGather", OP.bypass, replica_groups=groups,
                    ins=[g_src[l][ASPLIT:NC_N, :]], outs=[g_fullB[l][:]])

                # ================= phase B: gather + indicator matmul ======
                if stage_cap < 3:
                    continue
                sync_mode = bool(os.environ.get("GNN_SYNC"))
                if l == 0:
                    cum_calls = [0, 0, 0, 0]
                    prev_trigger = [None, None, None, None]
                    call_ctr = [0]
                emitted = {"A": 0, "B": 0}
                call_trig = {}
                arenas = {"A": arenaA, "B": arenaB}
                n_region_calls = {"A": meta["na_calls"], "B": meta["nb_calls"]}

                def emit_call(reg, k):
                    q = call_ctr[0] % 4
                    call_ctr[0] += 1
                    col0 = (0 if reg == "A" else meta["wA"]) + k * (CAP // 16)
                    in_view = g_fullA[l][:] if reg == "A" else g_fullB[l][:]
                    arena = arenas[reg]
                    slot0 = (8 * k) % ARENA_S
                    if sync_mode:
                        nc.gpsimd.dma_gather(
                            out_ap=arena[:, slot0:slot0 + 8, :],
                            in_ap=in_view,
                            idxs_ap=idx_sb[:, col0:col0 + CAP // 16],
                            num_idxs=CAP, num_idxs_reg=CAP,
                            elem_size=ROW, queue_num=q)
                        call_trig[(reg, k)] = None
                        return
                    gate = None
                    if cum_calls[q] > 0:
                        # full drain of queue q before the next prep: one
                        # call in flight per queue makes the cumulative
                        # 16-per-call thresholds exact (no engine skew)
                        gate = nc.gpsimd.wait_ge(dma_sems[q],
                                                 16 * cum_calls[q])
                        deps = bass.InstructionNameOrderedSet()
                        deps.add(prev_trigger[q].ins.name)
                        gate.ins.add_nosync_dependencies_from(deps)
                    p = nc.gpsimd.dma_gather(
                        out_ap=arena[:, slot0:slot0 + 8, :],
                        in_ap=in_view,
                        idxs_ap=idx_sb[:, col0:col0 + CAP // 16],
                        num_idxs=CAP, num_idxs_reg=CAP,
                        elem_size=ROW, queue_num=q,
                        prepare_only=True, sem=dma_sems[q])
                    cum_calls[q] += 1
                    nc._gnn_prep_targets[p.ins.name] = 16 * cum_calls[q]
                    deps = bass.InstructionNameOrderedSet()
                    have = False
                    if gate is not None:
                        deps.add(gate.ins.name)
                        have = True
                    if prev_trigger[q] is not None:
                        deps.add(prev_trigger[q].ins.name)
                        have = True
                    if have:
                        p.ins.add_nosync_dependencies_from(deps)
                    prev_trigger[q] = nc.gpsimd.trigger_dma(
                        count=None, queue_num=q)
                    call_trig[(reg, k)] = prev_trigger[q]

                for b in range(nb):
                    bl = blocks[b]
                    b0 = b * BLK
                    bw = min(BLK, NC_N - b0)
                    while emitted["A"] < bl["needA"]:
                        emit_call("A", emitted["A"])
                        emitted["A"] += 1
                    while emitted["B"] < bl["needB"]:
                        emit_call("B", emitted["B"])
                        emitted["B"] += 1
                    if stage_cap < 4:
                        continue
                    off = bl["dl_off"]
                    n_sub = bl["n_sub"]
                    ind = wp.tile([BLK, s_max * BLK], BF16, tag="ind", bufs=3)
                    nc.vector.tensor_tensor(
                        out=ind[:, 0:n_sub * BLK].rearrange("p (s i) -> p s i", i=BLK),
                        in0=dl_sb[:, off:off + n_sub][:, :, None]
                            .to_broadcast([BLK, n_sub, BLK]),
                        in1=wsb["iota"][:, None, :].to_broadcast([BLK, n_sub, BLK]),
                        op=OP.is_equal)
                    ps_blk = pb.tile([BLK, GVAL], F32, tag="blk", space="PSUM")
                    for j, (reg, s, e0, e1) in enumerate(bl["subs"]):
                        arena = arenas[reg]
                        mm = nc.tensor.matmul(out=ps_blk[:],
                                              lhsT=ind[:, j * BLK:(j + 1) * BLK],
                                              rhs=arena[:, s % ARENA_S, 0:GVAL],
                                              start=(j == 0), stop=(j == n_sub - 1))
                        tg = call_trig.get((reg, s // 8))
                        if mm is not None and tg is not None:
                            # scheduling-order (no-sync) edge: keep stage
                            # consumers after their call's trigger in the PE
                            # stream, else PE head-of-line blocks on data
                            # whose trigger hasn't dispatched yet
                            deps = bass.InstructionNameOrderedSet()
                            deps.add(tg.ins.name)
                            mm.ins.add_nosync_dependencies_from(deps)
                    sb = wp.tile([BLK, GVAL], F32, tag="sbblk")
                    nc.vector.tensor_copy(sb[:], ps_blk[:])
                    rec = wp.tile([BLK, H], F32, tag="rec")
                    nc.vector.tensor_scalar_add(rec[:], sb[:, H * F:GVAL], 1e-30)
                    nc.vector.reciprocal(rec[:], rec[:])
                    agg = wp.tile([BLK, F], F32, tag="agg")
                    tmp = wp.tile([BLK, F], F32, tag="tmp")
                    nc.vector.tensor_tensor(out=agg[:], in0=sb[:, 0:F],
                                            in1=rec[:, 0:1].to_broadcast([BLK, F]),
                                            op=OP.mult)
                    nc.vector.tensor_tensor(out=tmp[:], in0=sb[:, F:2 * F],
                                            in1=rec[:, 1:2].to_broadcast([BLK, F]),
                                            op=OP.mult)
                    nc.vector.tensor_add(out=agg[:], in0=agg[:], in1=tmp[:])
                    agg_bf = wp.tile([BLK, F], BF16, tag="aggbf")
                    nc.vector.tensor_copy(agg_bf[:], agg[:])
                    ps_t = pb.tile([BLK, BLK], BF16, tag="tp", space="PSUM")
                    nc.tensor.transpose(out=ps_t[:F, :], in_=agg_bf[:, :F],
                                        identity=wsb["identb"][:])
                    nc.vector.tensor_add(out=hT_res[:, b0:b0 + bw],
                                         in0=hT_res[:, b0:b0 + bw],
                                         in1=ps_t[:F, :bw])

                # ================= BatchNorm + ReLU ========================
                if stage_cap < 5:
                    continue
                nc.vector.reduce_sum(out=stats[:, 0:1], in_=hT_res[:, 0:NC_N],
                                     axis=mybir.AxisListType.X)
                half = (NC_N + 1) // 2
                nc.scalar.square(scr[:, 0:half], hT_res[:, 0:half])
                nc.vector.reduce_sum(out=stats[:, 1:2], in_=scr[:, 0:half],
                                     axis=mybir.AxisListType.X)
                nc.scalar.square(scr[:, 0:NC_N - half], hT_res[:, half:NC_N])
                nc.vector.reduce_sum(out=stats[:, 4:5], in_=scr[:, 0:NC_N - half],
                                     axis=mybir.AxisListType.X)
                nc.vector.tensor_add(out=stats[:, 1:2], in0=stats[:, 1:2],
                                     in1=stats[:, 4:5])
                nc.sync.dma_start(out=bn_src[l][:], in_=stats[:, 0:2])
                nc.gpsimd.collective_compute(
                    "AllReduce", OP.add, replica_groups=groups,
                    ins=[bn_src[l][:]], outs=[bn_out[l][:]])
                nc.sync.dma_start(out=stats[:, 2:4], in_=bn_out[l][:])
                nc.scalar.mul(bnsc[:, 0:1], stats[:, 2:3], 1.0 / N)
                nc.scalar.mul(bnsc[:, 1:2], stats[:, 3:4], 1.0 / N)
                nc.vector.tensor_tensor(out=bnsc[:, 2:3], in0=bnsc[:, 0:1],
                                        in1=bnsc[:, 0:1], op=OP.mult)
                nc.vector.tensor_tensor(out=bnsc[:, 2:3], in0=bnsc[:, 1:2],
                                        in1=bnsc[:, 2:3], op=OP.subtract)
                nc.vector.tensor_scalar_add(bnsc[:, 2:3], bnsc[:, 2:3], BN_EPS)
                nc.vector.reciprocal(bnsc[:, 3:4], bnsc[:, 2:3])
                nc.scalar.sqrt(bnsc[:, 4:5], bnsc[:, 3:4])
                nc.vector.tensor_tensor(out=bnsc[:, 5:6], in0=bnsc[:, 4:5],
                                        in1=wsb[f"bn{l}"][:, 0:1], op=OP.mult)
                nc.vector.tensor_tensor(out=bnsc[:, 6:7], in0=bnsc[:, 0:1],
                                        in1=bnsc[:, 5:6], op=OP.mult)
                nc.vector.tensor_tensor(out=bnsc[:, 6:7], in0=wsb[f"bn{l}"][:, 1:2],
                                        in1=bnsc[:, 6:7], op=OP.subtract)
                nc.scalar.activation(hT_act[:, 0:NC_N], hT_res[:, 0:NC_N],
                                     AF.Relu, bias=bnsc[:, 6:7],
                                     scale=bnsc[:, 5:6])

            # ================= head ========================================
            out_sb = hp.tile([3, NC_N], F32, tag="osb")
            for ci in range(n_chunks):
                c0 = ci * CHUNK
                cw = min(CHUNK, NC_N - c0)
                ps_o = pp.tile([3, CHUNK], F32, tag="s", space="PSUM")
                nc.tensor.matmul(out=ps_o[:, :cw], lhsT=wsb["headW"][:],
                                 rhs=hT_act[:F, c0:c0 + cw], start=True, stop=True)
                nc.scalar.activation(out_sb[:, c0:c0 + cw], ps_o[:, :cw],
                                     AF.Identity, bias=wsb["headb"][:, 0:1])
            nc.sync.dma_start(out=out_ext[:], in_=out_sb[:, 0:NC_N])

    return nc


# ---------------------------------------------------------------- run cache
_CACHE = {}


def _build_inputs(inputs, meta, idx_full, dl_dev):
    w = pack_weights(inputs)
    x = np.asarray(inputs["x"], np.float32)
    in_maps = []
    for c in range(N_CORES):
        m = dict(w)
        m["xT"] = np.ascontiguousarray(
            x[c * NC_N:(c + 1) * NC_N, :].T).astype(BF)
        m["idx"] = np.ascontiguousarray(idx_full[c])
        m["dstloc"] = np.ascontiguousarray(dl_dev[c])
        in_maps.append(m)
    return in_maps


def kernel(**inputs):
    from concourse.bass_utils import run_bass_kernel_spmd

    _install_hookshim()
    edge = np.asarray(inputs["edge_index"])
    key = hashlib.sha1(edge.tobytes()).hexdigest()
    if key not in _CACHE:
        idx_full, dl_dev, meta = preprocess(edge)
        nc = build_program(meta)
        nc.finalize()
        if not os.environ.get("GNN_SYNC"):
            n_remap, n_del = remap_dmasw_waits(nc)
            print(f"remapped DMASW waits on {n_remap} instructions, "
                  f"deleted {n_del} IncSwdgeSem")
        n_fix = legalize_waits(nc)
        if n_fix:
            print(f"legalize_waits fixed {n_fix} instructions post-finalize")
        _CACHE[key] = (idx_full, dl_dev, meta, nc)
    idx_full, dl_dev, meta, nc = _CACHE[key]
    in_maps = _build_inputs(inputs, meta, idx_full, dl_dev)
    res = run_bass_kernel_spmd(
        nc, in_maps, list(range(N_CORES)),
        trace=bool(os.environ.get("GNN_TRACE")))
    if res.exec_time_ns is not None:
        print(f"HW exec time: {res.exec_time_ns} ns")
    out = np.concatenate([res.results[c]["out"] for c in range(N_CORES)],
                         axis=1)  # [3, N]
    return np.ascontiguousarray(out.T).astype(np.float32)


# revision 39
# speedup vs baseline: 1.9513x; 1.6189x over previous
"""AttnGraphSAGE on 8 Trainium2 NeuronCores (Bass/Tile) — v2.

Math restructuring (unchanged from v1): attention logits depend only on the
SOURCE node, so the whole edge phase is ONE segment-sum over dst of per-src
rows G[n] = [E_0*x_jm_0 (64) | E_1*x_jm_1 (64) | E_0 | E_1] (130 values).

v2 performance changes:
  * G rows are bf16, 256-elem / 512B strides (was f32 768B): halves the
    random-gather HBM traffic and the AllGather volume.  All matmul operands
    (weights, activations, indicator) are bf16 -> 1-pass PE instead of 4.
  * dma_gather calls are PREPARE_ONLY + trigger_dma: GpSimd only generates
    descriptors (~1us/call) instead of blocking until the DMA lands
    (~7us/call serialized in v1).  DMA queues stay deep and overlap compute.
  * The G table is AllGather'd in TWO halves (A = rows [0,3072) of each
    core, B = rows [3072,6250)): AG(A) overlaps phase-A compute of the B
    rows, and each half has <32768 rows so the two gather address ranges
    double as the int16-index split (no separate lo/hi split needed).
  * Exact per-block index counts (padded only to the max across the 8 cores
    so the program stays SPMD-uniform), 0-padded: no trailing -1 indices,
    ~15% fewer descriptors than v1's global-max padding.
"""
import os
import sys
import types
import hashlib
import contextlib

sys.path.insert(0, "/opt/trn_rl_repo")

import numpy as np
import ml_dtypes

import concourse.bass as bass
import concourse.bacc as bacc
import concourse.mybir as mybir
from concourse import tile

# ---------------------------------------------------------------- constants
N = 50000
E = 800000
IN = 128
F = 64
H = 2
N_CORES = 8
NC_N = N // N_CORES          # 6250 nodes per core
BLK = 128                    # dst nodes per block
ROW = 256                    # G row stride in bf16 elems (512B)
GVAL = 2 * F + H             # 130 used cols
ASPLIT = 3072                # rows per core in AllGather half A
BSPLIT = NC_N - ASPLIT       # 3178 rows in half B
CHUNK = 512                  # phase-A node chunk (6 chunks cover ASPLIT)
CAP = int(os.environ.get("GNN_CAP", "1024"))   # idxs per gather call (HW max)
ARENA_S = 64                 # ring-arena subtiles per region (8 calls)
GATE_D = 4                   # calls in flight per queue (ring + sem-slot cap)
F32 = mybir.dt.float32
BF16 = mybir.dt.bfloat16
I16 = mybir.dt.int16
AF = mybir.ActivationFunctionType
OP = mybir.AluOpType
BN_EPS = 1e-5
LEAKY = 0.2
BF = ml_dtypes.bfloat16


# ------------------------------------------------------- axon profile shim
def _install_hookshim():
    if "antenv.axon_hooks" in sys.modules:
        return
    mod = types.ModuleType("antenv.axon_hooks")
    _h = [None]
    mod.set_axon_ntff_profile_hook = lambda h: _h.__setitem__(0, h)
    mod.get_axon_ntff_profile_hook = lambda: _h[0]
    try:
        import antenv
        sys.modules["antenv.axon_hooks"] = mod
        antenv.axon_hooks = mod
        from trn_agent_boot.trn_boot import _ntff_profile_via_ctypes
        mod.set_axon_ntff_profile_hook(
            _ntff_profile_via_ctypes("/opt/axon/libaxon_pjrt.so")
        )
    except Exception:
        pass


def remap_dmasw_waits(nc):
    """Remap waits on Tile's DMASW lane semaphores to the per-queue gather
    DMA-completion sems.

    Tile assigned each PREPARE_ONLY gather prep a DMASW lane (round-robin)
    and derived all downstream waits (consumers, ring flow control) as
    ``DMASW{lane} >= 16*tick``.  But the sem actually baked into the
    descriptors (and bumped by the SDMA engines) is our per-queue gdma sem,
    so those lane sems never move.  Each prep records its assigned
    (lane proc, tick); since each queue's ring is FIFO, the k-th prep of
    queue q has completed exactly when gdma{q} >= 16*k.  Rewrite every
    DMASW wait for (lane, tick) into the equivalent (and race-free)
    per-queue wait."""
    from concourse.tile_sem_assignment import PROC_NAME_TO_IDX
    inv_proc = {v: k for k, v in PROC_NAME_TO_IDX.items()}

    insts = []
    for func in nc.m.functions:
        for block in func.blocks:
            insts.extend(block.instructions)

    # (lane_name, 16*tick) -> (gdma sem id, gdma name, block-level target)
    lane_map = {}
    for inst in insts:
        if type(inst).__name__ == "InstDMAGatherAnt" and \
                getattr(inst, "gen_mode", 0) == 1:
            lane = inv_proc[inst.bass_scheduled_proc]
            upd = inst.sync_info.on_update[0]
            assert upd.ant_name.startswith("gdma"), upd.ant_name
            key = (lane, 16 * inst.bass_scheduled_tick)
            assert key not in lane_map, key
            lane_map[key] = (upd.id, upd.ant_name,
                             nc._gnn_prep_targets[inst.name])

    # waits with these prefixes are deferred from a prep to its trigger:
    # the prep only writes ring descriptors; the DMA (which actually touches
    # the arena / g_full) fires at the trigger, so enforcing reader-WAR and
    # collective deps there frees desc-gen to run ahead.
    XFER = ("PE_", "DVE_", "Act", "Collectives_")
    n = 0
    n_del = 0
    n_xfer = 0
    for func in nc.m.functions:
        for block in func.blocks:
            kept = []
            for inst in block.instructions:
                # Tile's per-prep DMASW shadow-sem maintenance is dead weight
                # once nothing uses the lane sems (1.65us of Pool each, plus
                # serializing ring-drain waits); the ring-capacity gates keep
                # the ring below capacity without it.
                if type(inst).__name__ == "InstIncSwdgeSem":
                    n_del += 1
                    continue
                kept.append(inst)
                si = inst.sync_info
                if not (si and si.on_wait):
                    continue
                changed = False
                new_waits = []
                trig = nc._gnn_prep_trig.get(inst.name)
                for w in si.on_wait:
                    if w.ant_name and w.ant_name.startswith("DMASW"):
                        lane = w.ant_name.split("_")[0]
                        sid, sname, thresh = lane_map[(lane, w.wait_value)]
                        new_waits.append(mybir.SyncWait(
                            sync_type="semaphore", id=sid,
                            wait_mode="sem-ge-imm",
                            wait_value=thresh, ant_name=sname))
                        changed = True
                    elif trig is not None and w.ant_name and \
                            w.ant_name.startswith(XFER):
                        tsi = trig.sync_info
                        tsi.on_wait = list(tsi.on_wait or []) + [w]
                        changed = True
                        n_xfer += 1
                    else:
                        new_waits.append(w)
                if changed:
                    si.on_wait = new_waits
                    n += 1
            block.instructions[:] = kept
    return n, n_del, n_xfer


# ------------------------------------------------------------ wait legalize
def legalize_waits(nc):
    """TRN2 TPB instructions have ONE sync-wait slot (EventSemaphore has 2);
    hoist extra waits left by the Tile scheduler into EVSEM prequels."""
    n_fixed = 0
    for func in nc.m.functions:
        for block in func.blocks:
            new_insts = []
            for inst in block.instructions:
                si = inst.sync_info
                waits = list(si.on_wait) if si and si.on_wait else []
                cap = 2 if isinstance(inst, mybir.InstEventSemaphore) else 1
                if isinstance(inst, mybir.InstDrain):
                    cap = 1
                if len(waits) > cap:
                    extra, keep = waits[:-cap], waits[-cap:]
                    for i in range(0, len(extra), 2):
                        new_insts.append(
                            mybir.InstEventSemaphore(
                                name=nc.get_next_instruction_name(),
                                ins=[],
                                outs=[],
                                engine=inst.engine,
                                sync_info=mybir.SyncInfo(
                                    on_wait=extra[i:i + 2], on_update=[]
                                ),
                            )
                        )
                    si.on_wait = keep
                    n_fixed += 1
                new_insts.append(inst)
            block.instructions[:] = new_insts
    return n_fixed


# ----------------------------------------------------------- host preprocess
def preprocess(edge_index):
    """Sort edges by dst, partition per core / per 128-dst block, split each
    block's edges into A/B-region runs (by source row within its owner core),
    pad counts to the per-block max across cores (program is SPMD-uniform).

    Each region's padded edge stream is then PACKED into gather calls of
    exactly CAP indices spanning block boundaries (the Q7 per-call fixed
    cost ~4us dominates, so call count is what matters).  Calls write 8
    consecutive subtiles of a 32-subtile ring arena per region; a block's
    indicator matmul consumes the (possibly boundary-shared) subtiles it
    touches, with foreign slots killed by dl=-1."""
    nb = (NC_N + BLK - 1) // BLK
    src = np.asarray(edge_index[0], np.int64)
    dst = np.asarray(edge_index[1], np.int64)
    order = np.argsort(dst, kind="stable")
    ds, ss = dst[order], src[order]

    core = ds // NC_N
    blk = (ds - core * NC_N) // BLK
    gblk = core * nb + blk
    n_gblk = N_CORES * nb
    bbounds = np.searchsorted(gblk, np.arange(n_gblk + 1))

    # source slot within the AllGather'd table halves
    sc = ss // NC_N
    r = ss - sc * NC_N
    in_a = r < ASPLIT
    slot = np.where(in_a, sc * ASPLIT + r, sc * BSPLIT + (r - ASPLIT))

    runs = {}    # (core, block) -> (a_slots, a_dl, b_slots, b_dl)
    n_a = np.zeros((N_CORES, nb), np.int64)
    n_b = np.zeros((N_CORES, nb), np.int64)
    for g in range(n_gblk):
        e0, e1 = bbounds[g], bbounds[g + 1]
        c, b = g // nb, g % nb
        base = c * NC_N + b * BLK
        sl, dl, ia = slot[e0:e1], ds[e0:e1] - base, in_a[e0:e1]
        a_s, a_d = sl[ia], dl[ia]
        b_s, b_d = sl[~ia], dl[~ia]
        # ascending slot order inside each run -> ascending HBM addresses
        oa, ob = np.argsort(a_s, kind="stable"), np.argsort(b_s, kind="stable")
        runs[(c, b)] = (a_s[oa], a_d[oa], b_s[ob], b_d[ob])
        n_a[c, b], n_b[c, b] = len(a_s), len(b_s)

    n_a_u = n_a.max(axis=0).astype(int)   # uniform (max-over-cores) counts
    n_b_u = n_b.max(axis=0).astype(int)

    # region stream layout: block b's run occupies [start[b], start[b]+n)
    def region_layout(n_u):
        starts = np.concatenate([[0], np.cumsum(n_u)])
        total = int(starts[-1])
        total_pad = (total + CAP - 1) // CAP * CAP   # pad last call
        n_calls = total_pad // CAP
        n_sub = total_pad // BLK
        return starts, total, total_pad, n_calls, n_sub

    sa_starts, sa_tot, sa_pad, na_calls, _ = region_layout(n_a_u)
    sb_starts, sb_tot, sb_pad, nb_calls, _ = region_layout(n_b_u)

    # per block: touched subtiles per region + dl columns
    blocks = []
    tot_s = 0
    for b in range(nb):
        entry = dict(dl_off=tot_s, subs=[])   # subs: (region, glob_subtile)
        for reg, starts, n_u in (("A", sa_starts, n_a_u), ("B", sb_starts, n_b_u)):
            e0, e1 = int(starts[b]), int(starts[b] + n_u[b])
            s0, s1 = e0 // BLK, (e1 + BLK - 1) // BLK
            for s in range(s0, s1):
                entry["subs"].append((reg, s, e0, e1))
            # calls needed (exclusive prefix): region call idx covering e1-1
            entry[f"need{reg}"] = (e1 + CAP - 1) // CAP if e1 > 0 else 0
        entry["n_sub"] = len(entry["subs"])
        tot_s += entry["n_sub"]
        blocks.append(entry)

    # index planes: region streams wrapped per call (CAP idx = CAP//16 cols)
    wA, wB = na_calls * (CAP // 16), nb_calls * (CAP // 16)
    w_idx = wA + wB
    idx_dev = np.zeros((N_CORES, 16, w_idx), np.int16)
    dl_dev = np.full((N_CORES, BLK, tot_s), -1.0, np.float32)

    for c in range(N_CORES):
        for reg, starts, n_u, pad_tot, col0 in (
                ("A", sa_starts, n_a_u, sa_pad, 0),
                ("B", sb_starts, n_b_u, sb_pad, wA)):
            streamv = np.zeros((pad_tot,), np.int64)
            for b in range(nb):
                a_s, a_d, b_s, b_d = runs[(c, b)]
                v = a_s if reg == "A" else b_s
                e0 = int(starts[b])
                streamv[e0:e0 + len(v)] = v
            # wrap16 whole region stream: idx i -> (p=i%16, col=i//16)
            idx_dev[c, :, col0:col0 + pad_tot // 16] = \
                streamv.reshape(-1, 16).T.astype(np.int16)
        # dl columns
        for b in range(nb):
            bl = blocks[b]
            a_s, a_d, b_s, b_d = runs[(c, b)]
            for k, (reg, s, e0, e1) in enumerate(bl["subs"]):
                dvals = a_d if reg == "A" else b_d
                base = e0
                col = np.full((BLK,), -1.0, np.float32)
                lo = max(e0, s * BLK)
                hi = min(e0 + len(dvals), (s + 1) * BLK)
                if hi > lo:
                    col[lo - s * BLK:hi - s * BLK] = dvals[lo - base:hi - base]
                dl_dev[c, :, bl["dl_off"] + k] = col

    idx_full = np.tile(idx_dev, (1, 8, 1))     # replicate to 128 partitions
    s_max = max(bl["n_sub"] for bl in blocks)
    meta = dict(nb=nb, blocks=blocks, w_idx=w_idx, tot_s=tot_s, s_max=s_max,
                na_calls=na_calls, nb_calls=nb_calls,
                sa_pad=sa_pad, sb_pad=sb_pad, wA=wA)
    return idx_full, dl_dev.astype(BF), meta


def pack_weights(inp):
    """Host-side packing of the small replicated weight tensors (bf16)."""
    def bd(av):  # [H, 2F] -> block-diag [H*F, H] halves (query, msg)
        av = np.asarray(av, np.float32)
        q = np.zeros((H * F, H), np.float32)
        m = np.zeros((H * F, H), np.float32)
        for h in range(H):
            q[h * F:(h + 1) * F, h] = av[h, :F]
            m[h * F:(h + 1) * F, h] = av[h, F:]
        return q, m

    w = {}
    for l in (0, 1):
        w[f"Wr{l}"] = np.asarray(inp[f"Wr{l}"], np.float32).astype(BF)
        w[f"Wn{l}"] = np.asarray(inp[f"Wn{l}"], np.float32).astype(BF)
        w[f"Wa{l}"] = np.asarray(inp[f"Wa{l}"], np.float32).astype(BF)
        q_, m_ = bd(inp[f"av{l}"])
        w[f"avq{l}"], w[f"avm{l}"] = q_.astype(BF), m_.astype(BF)
        w[f"bn{l}"] = np.stack(
            [np.asarray(inp[f"g{l}"], np.float32),
             np.asarray(inp[f"b{l}"], np.float32)], axis=1)  # [64,2] f32
    w["headW"] = np.asarray(inp["head_W"], np.float32).astype(BF)
    w["headb"] = np.asarray(inp["head_b"], np.float32).reshape(3, 1)
    w["iota"] = np.broadcast_to(np.arange(BLK, dtype=np.float32),
                                (BLK, BLK)).astype(BF)
    w["identb"] = np.eye(BLK, dtype=np.float32).astype(BF)
    w["identf"] = np.eye(BLK, dtype=np.float32)
    bo = np.zeros((H, H * F), np.float32)
    for h in range(H):
        bo[h, h * F:(h + 1) * F] = 1.0
    w["blkones"] = bo.astype(BF)
    return w


# ------------------------------------------------------------ device program
def build_program(meta):
    nb = meta["nb"]
    blocks = meta["blocks"]
    w_idx = meta["w_idx"]
    tot_s = meta["tot_s"]
    s_max = meta["s_max"]
    dims = [IN, F]

    nc = bacc.Bacc(None, num_swdge_queues=4)
    nc._gnn_prep_targets = {}   # prep inst name -> completion sem target
    nc._gnn_prep_trig = {}      # prep inst name -> its trigger (mybir inst)

    # ---- I/O
    xT = nc.declare_dram_parameter("xT", [IN, NC_N], BF16, isOutput=False)
    idx_in = nc.declare_dram_parameter("idx", [BLK, w_idx], I16, isOutput=False)
    dl_in = nc.declare_dram_parameter("dstloc", [BLK, tot_s], BF16, isOutput=False)
    wext = {}
    for l in (0, 1):
        d = dims[l]
        wext[f"Wr{l}"] = nc.declare_dram_parameter(f"Wr{l}", [d, F], BF16, isOutput=False)
        wext[f"Wn{l}"] = nc.declare_dram_parameter(f"Wn{l}", [d, H * F], BF16, isOutput=False)
        wext[f"Wa{l}"] = nc.declare_dram_parameter(f"Wa{l}", [d, H * F], BF16, isOutput=False)
        wext[f"avq{l}"] = nc.declare_dram_parameter(f"avq{l}", [H * F, H], BF16, isOutput=False)
        wext[f"avm{l}"] = nc.declare_dram_parameter(f"avm{l}", [H * F, H], BF16, isOutput=False)
        wext[f"bn{l}"] = nc.declare_dram_parameter(f"bn{l}", [F, 2], F32, isOutput=False)
    wext["headW"] = nc.declare_dram_parameter("headW", [F, 3], BF16, isOutput=False)
    wext["headb"] = nc.declare_dram_parameter("headb", [3, 1], F32, isOutput=False)
    wext["iota"] = nc.declare_dram_parameter("iota", [BLK, BLK], BF16, isOutput=False)
    wext["identb"] = nc.declare_dram_parameter("identb", [BLK, BLK], BF16, isOutput=False)
    wext["identf"] = nc.declare_dram_parameter("identf", [BLK, BLK], F32, isOutput=False)
    wext["blkones"] = nc.declare_dram_parameter("blkones", [H, H * F], BF16, isOutput=False)
    out_ext = nc.declare_dram_parameter("out", [3, NC_N], F32, isOutput=True)

    # ---- internal DRAM
    g_src = [nc.dram_tensor(f"g_src{l}", [NC_N, ROW], BF16) for l in (0, 1)]
    g_fullA = [nc.dram_tensor(f"g_fullA{l}", [N_CORES * ASPLIT, ROW], BF16,
                              addr_space="Shared") for l in (0, 1)]
    g_fullB = [nc.dram_tensor(f"g_fullB{l}", [N_CORES * BSPLIT, ROW], BF16,
                              addr_space="Shared") for l in (0, 1)]
    bn_src = [nc.dram_tensor(f"bn_src{l}", [F, 2], F32) for l in (0, 1)]
    bn_out = [nc.dram_tensor(f"bn_out{l}", [F, 2], F32, addr_space="Shared")
              for l in (0, 1)]
    groups = [list(range(N_CORES))]

    n_chunks = (NC_N + CHUNK - 1) // CHUNK
    a_chunks = ASPLIT // CHUNK       # chunks covering the A half exactly
    stage_cap = int(os.environ.get("GNN_STAGE", "9"))
    layer_cap = int(os.environ.get("GNN_LAYERS", "2"))

    # 8 rotating completion sems per queue: call k (per-queue ordinal) uses
    # slot k%8.  With <=8 calls in flight per queue (ring gates), each slot
    # has at most one call in flight, so the threshold 16*(k//8+1) is exact.
    dma_sems = [[nc.alloc_semaphore(f"gdma{q}_{j}") for j in range(8)]
                for q in range(4)]

    with tile.TileContext(nc) as tc:
        with contextlib.ExitStack() as ctx:
            cpool = ctx.enter_context(tc.tile_pool(name="const", bufs=1))
            wp = ctx.enter_context(tc.tile_pool(name="work", bufs=2))
            hp = ctx.enter_context(tc.tile_pool(name="resid", bufs=1))
            pp = ctx.enter_context(tc.tile_pool(name="psA", bufs=1, space="PSUM"))
            pb = ctx.enter_context(tc.tile_pool(name="psB", bufs=2, space="PSUM"))

            # ---- load constants
            wsb = {}
            for k, ext in wext.items():
                t = cpool.tile(list(ext.shape), ext.dtype, tag=k)
                nc.sync.dma_start(out=t[:], in_=ext[:])
                wsb[k] = t
            idx_sb = cpool.tile([BLK, w_idx], I16, tag="idx")
            nc.sync.dma_start(out=idx_sb[:], in_=idx_in[:])
            dl_sb = cpool.tile([BLK, tot_s], BF16, tag="dl")
            nc.sync.dma_start(out=dl_sb[:], in_=dl_in[:])

            hT_res = hp.tile([F, NC_N], F32, tag="hres")
            hT_act = hp.tile([F, NC_N], BF16, tag="hact")
            nc.vector.memset(hT_act[:], 0.0)
            arenaA = hp.tile([BLK, ARENA_S, ROW], BF16, tag="arA")
            arenaB = hp.tile([BLK, ARENA_S, ROW], BF16, tag="arB")
            scr = hp.tile([F, (NC_N + 1) // 2], F32, tag="scr")
            stats = hp.tile([F, 6], F32, tag="stats")
            bnsc = hp.tile([F, 8], F32, tag="bnsc")

            for l in (0, 1)[:layer_cap]:
                d = dims[l]
                # ================= phase A: per-node G rows + x_root =======
                for ci in range(n_chunks):
                    c0 = ci * CHUNK
                    cw = min(CHUNK, NC_N - c0)
                    if l == 0:
                        rhs = wp.tile([IN, CHUNK], BF16, tag="xchunk")
                        nc.sync.dma_start(out=rhs[:, :cw], in_=xT[:, c0:c0 + cw])
                        rhs_ap = rhs[:IN, :cw]
                    else:
                        rhs_ap = hT_act[:F, c0:c0 + cw]

                    ps_jm = pp.tile([H * F, CHUNK], F32, tag="jm", space="PSUM")
                    ps_iq = pp.tile([H * F, CHUNK], F32, tag="iq", space="PSUM")
                    ps_r = pp.tile([F, CHUNK], F32, tag="r", space="PSUM")
                    nc.tensor.matmul(out=ps_jm[:, :cw], lhsT=wsb[f"Wn{l}"][:d, :],
                                     rhs=rhs_ap, start=True, stop=True)
                    nc.tensor.matmul(out=ps_iq[:, :cw], lhsT=wsb[f"Wa{l}"][:d, :],
                                     rhs=rhs_ap, start=True, stop=True)
                    nc.tensor.matmul(out=ps_r[:, :cw], lhsT=wsb[f"Wr{l}"][:d, :],
                                     rhs=rhs_ap, start=True, stop=True)
                    nc.vector.tensor_copy(hT_res[:, c0:c0 + cw], ps_r[:, :cw])

                    jm = wp.tile([H * F, CHUNK], BF16, tag="jm_sb")
                    nc.scalar.activation(jm[:, :cw], ps_jm[:, :cw], AF.Identity)
                    # leaky(x) = max(x, 0.2x)
                    lkjm = wp.tile([H * F, CHUNK], BF16, tag="lkjm")
                    nc.scalar.mul(lkjm[:, :cw], ps_jm[:, :cw], LEAKY)
                    nc.vector.tensor_tensor(out=lkjm[:, :cw], in0=lkjm[:, :cw],
                                            in1=jm[:, :cw], op=OP.max)
                    iq = wp.tile([H * F, CHUNK], BF16, tag="iq_sb")
                    nc.scalar.activation(iq[:, :cw], ps_iq[:, :cw], AF.Identity)
                    lkiq = wp.tile([H * F, CHUNK], BF16, tag="lkiq")
                    nc.scalar.mul(lkiq[:, :cw], ps_iq[:, :cw], LEAKY)
                    nc.vector.tensor_tensor(out=lkiq[:, :cw], in0=lkiq[:, :cw],
                                            in1=iq[:, :cw], op=OP.max)
                    ps_s = pp.tile([H, CHUNK], F32, tag="s", space="PSUM")
                    nc.tensor.matmul(out=ps_s[:, :cw], lhsT=wsb[f"avq{l}"][:],
                                     rhs=lkiq[:, :cw], start=True, stop=False)
                    nc.tensor.matmul(out=ps_s[:, :cw], lhsT=wsb[f"avm{l}"][:],
                                     rhs=lkjm[:, :cw], start=False, stop=True)
                    e_sb = wp.tile([H, CHUNK], BF16, tag="esb")
                    nc.scalar.activation(e_sb[:, :cw], ps_s[:, :cw], AF.Exp)
                    # broadcast E over the per-head 64 features via matmul
                    ps_eb = pp.tile([H * F, CHUNK], F32, tag="iq", space="PSUM")
                    nc.tensor.matmul(out=ps_eb[:, :cw], lhsT=wsb["blkones"][:],
                                     rhs=e_sb[:, :cw], start=True, stop=True)
                    eb = wp.tile([H * F, CHUNK], BF16, tag="eb")
                    nc.scalar.activation(eb[:, :cw], ps_eb[:, :cw], AF.Identity)
                    y = wp.tile([H * F, CHUNK], BF16, tag="y")
                    nc.vector.tensor_tensor(out=y[:, :cw], in0=jm[:, :cw],
                                            in1=eb[:, :cw], op=OP.mult)
                    # write G rows (transpose to node-major)
                    for q in range(0, cw, BLK):
                        qw = min(BLK, cw - q)
                        ps_t = pb.tile([BLK, BLK], BF16, tag="tp", space="PSUM")
                        nc.tensor.transpose(out=ps_t[:qw, :], in_=y[:, q:q + qw],
                                            identity=wsb["identb"][:])
                        ps_e = pb.tile([BLK, BLK], BF16, tag="tp", space="PSUM")
                        nc.tensor.transpose(out=ps_e[:qw, :H], in_=e_sb[:, q:q + qw],
                                            identity=wsb["identb"][:H, :H])
                        gt = wp.tile([BLK, ROW], BF16, tag="gt")
                        nc.vector.tensor_copy(gt[:qw, 0:H * F], ps_t[:qw, :])
                        nc.vector.tensor_copy(gt[:qw, H * F:GVAL], ps_e[:qw, :H])
                        nc.sync.dma_start(
                            out=g_src[l][c0 + q:c0 + q + qw, :],
                            in_=gt[:qw, :])
                    # AllGather half A as soon as its rows are written
                    if ci == a_chunks - 1 and stage_cap >= 2:
                        nc.gpsimd.collective_compute(
                            "AllGather", OP.bypass, replica_groups=groups,
                            ins=[g_src[l][0:ASPLIT, :]], outs=[g_fullA[l][:]])

                if stage_cap < 2:
                    continue
                nc.gpsimd.collective_compute(
                    "AllGather", OP.bypass, replica_groups=groups,
                    ins=[g_src[l][ASPLIT:NC_N, :]], outs=[g_fullB[l][:]])

                # ================= phase B: gather + indicator matmul ======
                if stage_cap < 3:
                    continue
                sync_mode = bool(os.environ.get("GNN_SYNC"))
                if l == 0:
                    cum_calls = [0, 0, 0, 0]   # per-queue call ordinals
                    prev_prep = [None, None, None, None]
                    prev_trigger = [None, None, None, None]
                    trig_of = {}               # (q, ordinal) -> trigger inst
                    call_ctr = [0]
                emitted = {"A": 0, "B": 0}
                call_trig = {}
                arenas = {"A": arenaA, "B": arenaB}

                def chain(inst, *prevs):
                    deps = bass.InstructionNameOrderedSet()
                    have = False
                    for pv in prevs:
                        if pv is not None:
                            deps.add(pv.ins.name)
                            have = True
                    if have:
                        inst.ins.add_nosync_dependencies_from(deps)

                def emit_call(reg, k):
                    q = call_ctr[0] % 4
                    call_ctr[0] += 1
                    col0 = (0 if reg == "A" else meta["wA"]) + k * (CAP // 16)
                    in_view = g_fullA[l][:] if reg == "A" else g_fullB[l][:]
                    arena = arenas[reg]
                    slot0 = (8 * k) % ARENA_S
                    if sync_mode:
                        nc.gpsimd.dma_gather(
                            out_ap=arena[:, slot0:slot0 + 8, :],
                            in_ap=in_view,
                            idxs_ap=idx_sb[:, col0:col0 + CAP // 16],
                            num_idxs=CAP, num_idxs_reg=CAP,
                            elem_size=ROW, queue_num=q)
                        call_trig[(reg, k)] = None
                        return
                    ordinal = cum_calls[q]
                    slot = ordinal % 8
                    gate = None
                    if ordinal >= GATE_D:
                        # ring-capacity gate: call (ordinal-GATE_D) of this
                        # queue must be fully drained -> at most GATE_D calls
                        # (~260 descs/engine of the ring) in flight per
                        # queue, and the 8 sem slots stay unambiguous
                        og = ordinal - GATE_D
                        gate = nc.gpsimd.wait_ge(dma_sems[q][og % 8],
                                                 16 * (og // 8 + 1))
                        chain(gate, trig_of[(q, og)], prev_prep[q])
                    p = nc.gpsimd.dma_gather(
                        out_ap=arena[:, slot0:slot0 + 8, :],
                        in_ap=in_view,
                        idxs_ap=idx_sb[:, col0:col0 + CAP // 16],
                        num_idxs=CAP, num_idxs_reg=CAP,
                        elem_size=ROW, queue_num=q,
                        prepare_only=True, sem=dma_sems[q][slot])
                    nc._gnn_prep_targets[p.ins.name] = 16 * (ordinal // 8 + 1)
                    chain(p, gate, prev_prep[q])
                    prev_prep[q] = p
                    t = nc.gpsimd.trigger_dma(count=1, queue_num=q)
                    chain(t, p, prev_trigger[q])
                    nc._gnn_prep_trig[p.ins.name] = t.ins
                    prev_trigger[q] = t
                    trig_of[(q, ordinal)] = t
                    call_trig[(reg, k)] = t
                    cum_calls[q] = ordinal + 1

                for b in range(nb):
                    bl = blocks[b]
                    b0 = b * BLK
                    bw = min(BLK, NC_N - b0)
                    while emitted["A"] < bl["needA"]:
                        emit_call("A", emitted["A"])
                        emitted["A"] += 1
                    while emitted["B"] < bl["needB"]:
                        emit_call("B", emitted["B"])
                        emitted["B"] += 1
                    if stage_cap < 4:
                        continue
                    off = bl["dl_off"]
                    n_sub = bl["n_sub"]
                    ind = wp.tile([BLK, s_max * BLK], BF16, tag="ind", bufs=3)
                    nc.vector.tensor_tensor(
                        out=ind[:, 0:n_sub * BLK].rearrange("p (s i) -> p s i", i=BLK),
                        in0=dl_sb[:, off:off + n_sub][:, :, None]
                            .to_broadcast([BLK, n_sub, BLK]),
                        in1=wsb["iota"][:, None, :].to_broadcast([BLK, n_sub, BLK]),
                        op=OP.is_equal)
                    ps_blk = pb.tile([BLK, GVAL], F32, tag="blk", space="PSUM")
                    for j, (reg, s, e0, e1) in enumerate(bl["subs"]):
                        arena = arenas[reg]
                        mm = nc.tensor.matmul(out=ps_blk[:],
                                              lhsT=ind[:, j * BLK:(j + 1) * BLK],
                                              rhs=arena[:, s % ARENA_S, 0:GVAL],
                                              start=(j == 0), stop=(j == n_sub - 1))
                        tg = call_trig.get((reg, s // 8))
                        if mm is not None and tg is not None:
                            # scheduling-order (no-sync) edge: keep stage
                            # consumers after their call's trigger in the PE
                            # stream, else PE head-of-line blocks on data
                            # whose trigger hasn't dispatched yet
                            deps = bass.InstructionNameOrderedSet()
                            deps.add(tg.ins.name)
                            mm.ins.add_nosync_dependencies_from(deps)
                    sb = wp.tile([BLK, GVAL], F32, tag="sbblk")
                    nc.vector.tensor_copy(sb[:], ps_blk[:])
                    rec = wp.tile([BLK, H], F32, tag="rec")
                    nc.vector.tensor_scalar_add(rec[:], sb[:, H * F:GVAL], 1e-30)
                    nc.vector.reciprocal(rec[:], rec[:])
                    agg = wp.tile([BLK, F], F32, tag="agg")
                    tmp = wp.tile([BLK, F], F32, tag="tmp")
                    nc.vector.tensor_tensor(out=agg[:], in0=sb[:, 0:F],
                                            in1=rec[:, 0:1].to_broadcast([BLK, F]),
                                            op=OP.mult)
                    nc.vector.tensor_tensor(out=tmp[:], in0=sb[:, F:2 * F],
                                            in1=rec[:, 1:2].to_broadcast([BLK, F]),
                                            op=OP.mult)
                    nc.vector.tensor_add(out=agg[:], in0=agg[:], in1=tmp[:])
                    agg_bf = wp.tile([BLK, F], BF16, tag="aggbf")
                    nc.vector.tensor_copy(agg_bf[:], agg[:])
                    ps_t = pb.tile([BLK, BLK], BF16, tag="tp", space="PSUM")
                    nc.tensor.transpose(out=ps_t[:F, :], in_=agg_bf[:, :F],
                                        identity=wsb["identb"][:])
                    nc.vector.tensor_add(out=hT_res[:, b0:b0 + bw],
                                         in0=hT_res[:, b0:b0 + bw],
                                         in1=ps_t[:F, :bw])

                # ================= BatchNorm + ReLU ========================
                if stage_cap < 5:
                    continue
                nc.vector.reduce_sum(out=stats[:, 0:1], in_=hT_res[:, 0:NC_N],
                                     axis=mybir.AxisListType.X)
                half = (NC_N + 1) // 2
                nc.scalar.square(scr[:, 0:half], hT_res[:, 0:half])
                nc.vector.reduce_sum(out=stats[:, 1:2], in_=scr[:, 0:half],
                                     axis=mybir.AxisListType.X)
                nc.scalar.square(scr[:, 0:NC_N - half], hT_res[:, half:NC_N])
                nc.vector.reduce_sum(out=stats[:, 4:5], in_=scr[:, 0:NC_N - half],
                                     axis=mybir.AxisListType.X)
                nc.vector.tensor_add(out=stats[:, 1:2], in0=stats[:, 1:2],
                                     in1=stats[:, 4:5])
                nc.sync.dma_start(out=bn_src[l][:], in_=stats[:, 0:2])
                nc.gpsimd.collective_compute(
                    "AllReduce", OP.add, replica_groups=groups,
                    ins=[bn_src[l][:]], outs=[bn_out[l][:]])
                nc.sync.dma_start(out=stats[:, 2:4], in_=bn_out[l][:])
                nc.scalar.mul(bnsc[:, 0:1], stats[:, 2:3], 1.0 / N)
                nc.scalar.mul(bnsc[:, 1:2], stats[:, 3:4], 1.0 / N)
                nc.vector.tensor_tensor(out=bnsc[:, 2:3], in0=bnsc[:, 0:1],
                                        in1=bnsc[:, 0:1], op=OP.mult)
                nc.vector.tensor_tensor(out=bnsc[:, 2:3], in0=bnsc[:, 1:2],
                                        in1=bnsc[:, 2:3], op=OP.subtract)
                nc.vector.tensor_scalar_add(bnsc[:, 2:3], bnsc[:, 2:3], BN_EPS)
                nc.vector.reciprocal(bnsc[:, 3:4], bnsc[:, 2:3])
                nc.scalar.sqrt(bnsc[:, 4:5], bnsc[:, 3:4])
                nc.vector.tensor_tensor(out=bnsc[:, 5:6], in0=bnsc[:, 4:5],
                                        in1=wsb[f"bn{l}"][:, 0:1], op=OP.mult)
                nc.vector.tensor_tensor(out=bnsc[:, 6:7], in0=bnsc[:, 0:1],
                                        in1=bnsc[:, 5:6], op=OP.mult)
                nc.vector.tensor_tensor(out=bnsc[:, 6:7], in0=wsb[f"bn{l}"][:, 1:2],
                                        in1=bnsc[:, 6:7], op=OP.subtract)
                nc.scalar.activation(hT_act[:, 0:NC_N], hT_res[:, 0:NC_N],
                                     AF.Relu, bias=bnsc[:, 6:7],
                                     scale=bnsc[:, 5:6])

            # ================= head ========================================
            for ci in range(n_chunks):
                c0 = ci * CHUNK
                cw = min(CHUNK, NC_N - c0)
                ps_o = pp.tile([3, CHUNK], F32, tag="s", space="PSUM")
                nc.tensor.matmul(out=ps_o[:, :cw], lhsT=wsb["headW"][:],
                                 rhs=hT_act[:F, c0:c0 + cw], start=True, stop=True)
                osb = wp.tile([3, CHUNK], F32, tag="osb")
                nc.scalar.activation(osb[:, :cw], ps_o[:, :cw],
                                     AF.Identity, bias=wsb["headb"][:, 0:1])
                nc.sync.dma_start(out=out_ext[:, c0:c0 + cw], in_=osb[:, :cw])

    return nc


# ---------------------------------------------------------------- run cache
_CACHE = {}


def _build_inputs(inputs, meta, idx_full, dl_dev):
    w = pack_weights(inputs)
    x = np.asarray(inputs["x"], np.float32)
    in_maps = []
    for c in range(N_CORES):
        m = dict(w)
        m["xT"] = np.ascontiguousarray(
            x[c * NC_N:(c + 1) * NC_N, :].T).astype(BF)
        m["idx"] = np.ascontiguousarray(idx_full[c])
        m["dstloc"] = np.ascontiguousarray(dl_dev[c])
        in_maps.append(m)
    return in_maps


def kernel(**inputs):
    from concourse.bass_utils import run_bass_kernel_spmd

    _install_hookshim()
    edge = np.asarray(inputs["edge_index"])
    key = hashlib.sha1(edge.tobytes()).hexdigest()
    if key not in _CACHE:
        idx_full, dl_dev, meta = preprocess(edge)
        nc = build_program(meta)
        nc.finalize()
        if not os.environ.get("GNN_SYNC"):
            n_remap, n_del, n_xfer = remap_dmasw_waits(nc)
            print(f"remapped DMASW waits on {n_remap} insts, deleted "
                  f"{n_del} IncSwdgeSem, moved {n_xfer} waits to triggers")
        n_fix = legalize_waits(nc)
        if n_fix:
            print(f"legalize_waits fixed {n_fix} instructions post-finalize")
        _CACHE[key] = (idx_full, dl_dev, meta, nc)
    idx_full, dl_dev, meta, nc = _CACHE[key]
    in_maps = _build_inputs(inputs, meta, idx_full, dl_dev)
    res = run_bass_kernel_spmd(
        nc, in_maps, list(range(N_CORES)),
        trace=bool(os.environ.get("GNN_TRACE")))
    if res.exec_time_ns is not None:
        print(f"HW exec time: {res.exec_time_ns} ns")
    out = np.concatenate([res.results[c]["out"] for c in range(N_CORES)],
                         axis=1)  # [3, N]
    return np.ascontiguousarray(out.T).astype(np.float32)


# revision 42
# speedup vs baseline: 2.4503x; 1.2557x over previous
"""AttnGraphSAGE on 8 Trainium2 NeuronCores (Bass/Tile) — v2.

Math restructuring (unchanged from v1): attention logits depend only on the
SOURCE node, so the whole edge phase is ONE segment-sum over dst of per-src
rows G[n] = [E_0*x_jm_0 (64) | E_1*x_jm_1 (64) | E_0 | E_1] (130 values).

v2 performance changes:
  * G rows are bf16, 256-elem / 512B strides (was f32 768B): halves the
    random-gather HBM traffic and the AllGather volume.  All matmul operands
    (weights, activations, indicator) are bf16 -> 1-pass PE instead of 4.
  * dma_gather calls are PREPARE_ONLY + trigger_dma: GpSimd only generates
    descriptors (~1us/call) instead of blocking until the DMA lands
    (~7us/call serialized in v1).  DMA queues stay deep and overlap compute.
  * The G table is AllGather'd in TWO halves (A = rows [0,3072) of each
    core, B = rows [3072,6250)): AG(A) overlaps phase-A compute of the B
    rows, and each half has <32768 rows so the two gather address ranges
    double as the int16-index split (no separate lo/hi split needed).
  * Exact per-block index counts (padded only to the max across the 8 cores
    so the program stays SPMD-uniform), 0-padded: no trailing -1 indices,
    ~15% fewer descriptors than v1's global-max padding.
"""
import os
import sys
import types
import hashlib
import contextlib

sys.path.insert(0, "/opt/trn_rl_repo")

import numpy as np
import ml_dtypes

import concourse.bass as bass
import concourse.bacc as bacc
import concourse.mybir as mybir
from concourse import tile

# ---------------------------------------------------------------- constants
N = 50000
E = 800000
IN = 128
F = 64
H = 2
N_CORES = 8
NC_N = N // N_CORES          # 6250 nodes per core
BLK = 128                    # dst nodes per block
ROW = 256                    # G row stride in bf16 elems (512B)
GVAL = 2 * F + H             # 130 used cols
ASPLIT = 3072                # rows per core in AllGather half A
BSPLIT = NC_N - ASPLIT       # 3178 rows in half B
CHUNK = 512                  # phase-A node chunk (6 chunks cover ASPLIT)
CAP = int(os.environ.get("GNN_CAP", "1024"))   # idxs per gather call (HW max)
ARENA_S = 64                 # ring-arena subtiles per region (8 calls)
GATE_D = 4                   # calls in flight per queue (ring + sem-slot cap)
F32 = mybir.dt.float32
BF16 = mybir.dt.bfloat16
I16 = mybir.dt.int16
AF = mybir.ActivationFunctionType
OP = mybir.AluOpType
BN_EPS = 1e-5
LEAKY = 0.2
BF = ml_dtypes.bfloat16


# ------------------------------------------------------- axon profile shim
def _install_hookshim():
    if "antenv.axon_hooks" in sys.modules:
        return
    mod = types.ModuleType("antenv.axon_hooks")
    _h = [None]
    mod.set_axon_ntff_profile_hook = lambda h: _h.__setitem__(0, h)
    mod.get_axon_ntff_profile_hook = lambda: _h[0]
    try:
        import antenv
        sys.modules["antenv.axon_hooks"] = mod
        antenv.axon_hooks = mod
        from trn_agent_boot.trn_boot import _ntff_profile_via_ctypes
        mod.set_axon_ntff_profile_hook(
            _ntff_profile_via_ctypes("/opt/axon/libaxon_pjrt.so")
        )
    except Exception:
        pass


def remap_dmasw_waits(nc):
    """Remap waits on Tile's DMASW lane semaphores to the per-queue gather
    DMA-completion sems.

    Tile assigned each PREPARE_ONLY gather prep a DMASW lane (round-robin)
    and derived all downstream waits (consumers, ring flow control) as
    ``DMASW{lane} >= 16*tick``.  But the sem actually baked into the
    descriptors (and bumped by the SDMA engines) is our per-queue gdma sem,
    so those lane sems never move.  Each prep records its assigned
    (lane proc, tick); since each queue's ring is FIFO, the k-th prep of
    queue q has completed exactly when gdma{q} >= 16*k.  Rewrite every
    DMASW wait for (lane, tick) into the equivalent (and race-free)
    per-queue wait."""
    from concourse.tile_sem_assignment import PROC_NAME_TO_IDX
    inv_proc = {v: k for k, v in PROC_NAME_TO_IDX.items()}

    insts = []
    for func in nc.m.functions:
        for block in func.blocks:
            insts.extend(block.instructions)

    # (lane_name, 16*tick) -> (gdma sem id, gdma name, block-level target)
    lane_map = {}
    for inst in insts:
        if type(inst).__name__ == "InstDMAGatherAnt" and \
                getattr(inst, "gen_mode", 0) == 1:
            lane = inv_proc[inst.bass_scheduled_proc]
            upd = inst.sync_info.on_update[0]
            assert upd.ant_name.startswith("gdma"), upd.ant_name
            key = (lane, 16 * inst.bass_scheduled_tick)
            assert key not in lane_map, key
            lane_map[key] = (upd.id, upd.ant_name,
                             nc._gnn_prep_targets[inst.name])

    # waits with these prefixes are deferred from a prep to its trigger:
    # the prep only writes ring descriptors; the DMA (which actually touches
    # the arena / g_full) fires at the trigger, so enforcing reader-WAR and
    # collective deps there frees desc-gen to run ahead.
    XFER = ("PE_", "DVE_", "Act", "Collectives_")
    n = 0
    n_del = 0
    n_xfer = 0
    for func in nc.m.functions:
        for block in func.blocks:
            kept = []
            for inst in block.instructions:
                # Tile's per-prep DMASW shadow-sem maintenance is dead weight
                # once nothing uses the lane sems (1.65us of Pool each, plus
                # serializing ring-drain waits); the ring-capacity gates keep
                # the ring below capacity without it.
                if type(inst).__name__ == "InstIncSwdgeSem":
                    n_del += 1
                    continue
                kept.append(inst)
                si = inst.sync_info
                if not (si and si.on_wait):
                    continue
                changed = False
                new_waits = []
                trig = nc._gnn_prep_trig.get(inst.name)
                for w in si.on_wait:
                    if w.ant_name and w.ant_name.startswith("DMASW"):
                        lane = w.ant_name.split("_")[0]
                        sid, sname, thresh = lane_map[(lane, w.wait_value)]
                        new_waits.append(mybir.SyncWait(
                            sync_type="semaphore", id=sid,
                            wait_mode="sem-ge-imm",
                            wait_value=thresh, ant_name=sname))
                        changed = True
                    elif trig is not None and w.ant_name and \
                            w.ant_name.startswith(XFER):
                        tsi = trig.sync_info
                        tsi.on_wait = list(tsi.on_wait or []) + [w]
                        changed = True
                        n_xfer += 1
                    else:
                        new_waits.append(w)
                if changed:
                    si.on_wait = new_waits
                    n += 1
            block.instructions[:] = kept
    return n, n_del, n_xfer


# ------------------------------------------------------------ wait legalize
def legalize_waits(nc):
    """TRN2 TPB instructions have ONE sync-wait slot (EventSemaphore has 2);
    hoist extra waits left by the Tile scheduler into EVSEM prequels."""
    n_fixed = 0
    for func in nc.m.functions:
        for block in func.blocks:
            new_insts = []
            for inst in block.instructions:
                si = inst.sync_info
                waits = list(si.on_wait) if si and si.on_wait else []
                cap = 2 if isinstance(inst, mybir.InstEventSemaphore) else 1
                if isinstance(inst, mybir.InstDrain):
                    cap = 1
                if len(waits) > cap:
                    extra, keep = waits[:-cap], waits[-cap:]
                    for i in range(0, len(extra), 2):
                        new_insts.append(
                            mybir.InstEventSemaphore(
                                name=nc.get_next_instruction_name(),
                                ins=[],
                                outs=[],
                                engine=inst.engine,
                                sync_info=mybir.SyncInfo(
                                    on_wait=extra[i:i + 2], on_update=[]
                                ),
                            )
                        )
                    si.on_wait = keep
                    n_fixed += 1
                new_insts.append(inst)
            block.instructions[:] = new_insts
    return n_fixed


# ----------------------------------------------------------- host preprocess
def preprocess(edge_index):
    """Sort edges by dst, partition per core / per 128-dst block, split each
    block's edges into A/B-region runs (by source row within its owner core),
    pad counts to the per-block max across cores (program is SPMD-uniform).

    Each region's padded edge stream is then PACKED into gather calls of
    exactly CAP indices spanning block boundaries (the Q7 per-call fixed
    cost ~4us dominates, so call count is what matters).  Calls write 8
    consecutive subtiles of a 32-subtile ring arena per region; a block's
    indicator matmul consumes the (possibly boundary-shared) subtiles it
    touches, with foreign slots killed by dl=-1."""
    nb = (NC_N + BLK - 1) // BLK
    src = np.asarray(edge_index[0], np.int64)
    dst = np.asarray(edge_index[1], np.int64)
    order = np.argsort(dst, kind="stable")
    ds, ss = dst[order], src[order]

    core = ds // NC_N
    blk = (ds - core * NC_N) // BLK
    gblk = core * nb + blk
    n_gblk = N_CORES * nb
    bbounds = np.searchsorted(gblk, np.arange(n_gblk + 1))

    # source slot within the AllGather'd table halves
    sc = ss // NC_N
    r = ss - sc * NC_N
    in_a = r < ASPLIT
    slot = np.where(in_a, sc * ASPLIT + r, sc * BSPLIT + (r - ASPLIT))

    runs = {}    # (core, block) -> (a_slots, a_dl, b_slots, b_dl)
    n_a = np.zeros((N_CORES, nb), np.int64)
    n_b = np.zeros((N_CORES, nb), np.int64)
    for g in range(n_gblk):
        e0, e1 = bbounds[g], bbounds[g + 1]
        c, b = g // nb, g % nb
        base = c * NC_N + b * BLK
        sl, dl, ia = slot[e0:e1], ds[e0:e1] - base, in_a[e0:e1]
        a_s, a_d = sl[ia], dl[ia]
        b_s, b_d = sl[~ia], dl[~ia]
        # ascending slot order inside each run -> ascending HBM addresses
        oa, ob = np.argsort(a_s, kind="stable"), np.argsort(b_s, kind="stable")
        runs[(c, b)] = (a_s[oa], a_d[oa], b_s[ob], b_d[ob])
        n_a[c, b], n_b[c, b] = len(a_s), len(b_s)

    n_a_u = n_a.max(axis=0).astype(int)   # uniform (max-over-cores) counts
    n_b_u = n_b.max(axis=0).astype(int)

    # region stream layout: block b's run occupies [start[b], start[b]+n)
    def region_layout(n_u):
        starts = np.concatenate([[0], np.cumsum(n_u)])
        total = int(starts[-1])
        total_pad = (total + CAP - 1) // CAP * CAP   # pad last call
        n_calls = total_pad // CAP
        n_sub = total_pad // BLK
        return starts, total, total_pad, n_calls, n_sub

    sa_starts, sa_tot, sa_pad, na_calls, _ = region_layout(n_a_u)
    sb_starts, sb_tot, sb_pad, nb_calls, _ = region_layout(n_b_u)

    # per block: touched subtiles per region + dl columns
    blocks = []
    tot_s = 0
    for b in range(nb):
        entry = dict(dl_off=tot_s, subs=[])   # subs: (region, glob_subtile)
        for reg, starts, n_u in (("A", sa_starts, n_a_u), ("B", sb_starts, n_b_u)):
            e0, e1 = int(starts[b]), int(starts[b] + n_u[b])
            s0, s1 = e0 // BLK, (e1 + BLK - 1) // BLK
            for s in range(s0, s1):
                entry["subs"].append((reg, s, e0, e1))
            # calls needed (exclusive prefix): region call idx covering e1-1
            entry[f"need{reg}"] = (e1 + CAP - 1) // CAP if e1 > 0 else 0
        entry["n_sub"] = len(entry["subs"])
        tot_s += entry["n_sub"]
        blocks.append(entry)

    # index planes: region streams wrapped per call (CAP idx = CAP//16 cols)
    wA, wB = na_calls * (CAP // 16), nb_calls * (CAP // 16)
    w_idx = wA + wB
    idx_dev = np.zeros((N_CORES, 16, w_idx), np.int16)
    dl_dev = np.full((N_CORES, BLK, tot_s), -1.0, np.float32)

    for c in range(N_CORES):
        for reg, starts, n_u, pad_tot, col0 in (
                ("A", sa_starts, n_a_u, sa_pad, 0),
                ("B", sb_starts, n_b_u, sb_pad, wA)):
            streamv = np.zeros((pad_tot,), np.int64)
            for b in range(nb):
                a_s, a_d, b_s, b_d = runs[(c, b)]
                v = a_s if reg == "A" else b_s
                e0 = int(starts[b])
                streamv[e0:e0 + len(v)] = v
            # wrap16 whole region stream: idx i -> (p=i%16, col=i//16)
            idx_dev[c, :, col0:col0 + pad_tot // 16] = \
                streamv.reshape(-1, 16).T.astype(np.int16)
        # dl columns
        for b in range(nb):
            bl = blocks[b]
            a_s, a_d, b_s, b_d = runs[(c, b)]
            for k, (reg, s, e0, e1) in enumerate(bl["subs"]):
                dvals = a_d if reg == "A" else b_d
                base = e0
                col = np.full((BLK,), -1.0, np.float32)
                lo = max(e0, s * BLK)
                hi = min(e0 + len(dvals), (s + 1) * BLK)
                if hi > lo:
                    col[lo - s * BLK:hi - s * BLK] = dvals[lo - base:hi - base]
                dl_dev[c, :, bl["dl_off"] + k] = col

    idx_full = np.tile(idx_dev, (1, 8, 1))     # replicate to 128 partitions
    s_max = max(bl["n_sub"] for bl in blocks)
    meta = dict(nb=nb, blocks=blocks, w_idx=w_idx, tot_s=tot_s, s_max=s_max,
                na_calls=na_calls, nb_calls=nb_calls,
                sa_pad=sa_pad, sb_pad=sb_pad, wA=wA)
    return idx_full, dl_dev.astype(BF), meta


def pack_weights(inp):
    """Host-side packing of the small replicated weight tensors (bf16)."""
    def bd(av):  # [H, 2F] -> block-diag [H*F, H] halves (query, msg)
        av = np.asarray(av, np.float32)
        q = np.zeros((H * F, H), np.float32)
        m = np.zeros((H * F, H), np.float32)
        for h in range(H):
            q[h * F:(h + 1) * F, h] = av[h, :F]
            m[h * F:(h + 1) * F, h] = av[h, F:]
        return q, m

    w = {}
    for l in (0, 1):
        w[f"Wr{l}"] = np.asarray(inp[f"Wr{l}"], np.float32).astype(BF)
        w[f"Wn{l}"] = np.asarray(inp[f"Wn{l}"], np.float32).astype(BF)
        w[f"Wa{l}"] = np.asarray(inp[f"Wa{l}"], np.float32).astype(BF)
        q_, m_ = bd(inp[f"av{l}"])
        w[f"avq{l}"], w[f"avm{l}"] = q_.astype(BF), m_.astype(BF)
        w[f"bn{l}"] = np.stack(
            [np.asarray(inp[f"g{l}"], np.float32),
             np.asarray(inp[f"b{l}"], np.float32)], axis=1)  # [64,2] f32
    w["headW"] = np.asarray(inp["head_W"], np.float32).astype(BF)
    w["headb"] = np.asarray(inp["head_b"], np.float32).reshape(3, 1)
    w["iota"] = np.broadcast_to(np.arange(BLK, dtype=np.float32),
                                (BLK, BLK)).astype(BF)
    w["identb"] = np.eye(BLK, dtype=np.float32).astype(BF)
    w["identf"] = np.eye(BLK, dtype=np.float32)
    bo = np.zeros((H, H * F), np.float32)
    for h in range(H):
        bo[h, h * F:(h + 1) * F] = 1.0
    w["blkones"] = bo.astype(BF)
    return w


# ------------------------------------------------------------ device program
def build_program(meta):
    nb = meta["nb"]
    blocks = meta["blocks"]
    w_idx = meta["w_idx"]
    tot_s = meta["tot_s"]
    s_max = meta["s_max"]
    dims = [IN, F]

    nc = bacc.Bacc(None, num_swdge_queues=4)
    nc._gnn_prep_targets = {}   # prep inst name -> completion sem target
    nc._gnn_prep_trig = {}      # prep inst name -> its trigger (mybir inst)

    # ---- I/O
    xT = nc.declare_dram_parameter("xT", [IN, NC_N], BF16, isOutput=False)
    idx_in = nc.declare_dram_parameter("idx", [BLK, w_idx], I16, isOutput=False)
    dl_in = nc.declare_dram_parameter("dstloc", [BLK, tot_s], BF16, isOutput=False)
    wext = {}
    for l in (0, 1):
        d = dims[l]
        wext[f"Wr{l}"] = nc.declare_dram_parameter(f"Wr{l}", [d, F], BF16, isOutput=False)
        wext[f"Wn{l}"] = nc.declare_dram_parameter(f"Wn{l}", [d, H * F], BF16, isOutput=False)
        wext[f"Wa{l}"] = nc.declare_dram_parameter(f"Wa{l}", [d, H * F], BF16, isOutput=False)
        wext[f"avq{l}"] = nc.declare_dram_parameter(f"avq{l}", [H * F, H], BF16, isOutput=False)
        wext[f"avm{l}"] = nc.declare_dram_parameter(f"avm{l}", [H * F, H], BF16, isOutput=False)
        wext[f"bn{l}"] = nc.declare_dram_parameter(f"bn{l}", [F, 2], F32, isOutput=False)
    wext["headW"] = nc.declare_dram_parameter("headW", [F, 3], BF16, isOutput=False)
    wext["headb"] = nc.declare_dram_parameter("headb", [3, 1], F32, isOutput=False)
    wext["iota"] = nc.declare_dram_parameter("iota", [BLK, BLK], BF16, isOutput=False)
    wext["identb"] = nc.declare_dram_parameter("identb", [BLK, BLK], BF16, isOutput=False)
    wext["identf"] = nc.declare_dram_parameter("identf", [BLK, BLK], F32, isOutput=False)
    wext["blkones"] = nc.declare_dram_parameter("blkones", [H, H * F], BF16, isOutput=False)
    out_ext = nc.declare_dram_parameter("out", [3, NC_N], F32, isOutput=True)

    # ---- internal DRAM
    g_src = [nc.dram_tensor(f"g_src{l}", [NC_N, ROW], BF16) for l in (0, 1)]
    g_fullA = [nc.dram_tensor(f"g_fullA{l}", [N_CORES * ASPLIT, ROW], BF16,
                              addr_space="Shared") for l in (0, 1)]
    g_fullB = [nc.dram_tensor(f"g_fullB{l}", [N_CORES * BSPLIT, ROW], BF16,
                              addr_space="Shared") for l in (0, 1)]
    bn_src = [nc.dram_tensor(f"bn_src{l}", [F, 2], F32) for l in (0, 1)]
    bn_out = [nc.dram_tensor(f"bn_out{l}", [F, 2], F32, addr_space="Shared")
              for l in (0, 1)]
    groups = [list(range(N_CORES))]

    n_chunks = (NC_N + CHUNK - 1) // CHUNK
    a_chunks = ASPLIT // CHUNK       # chunks covering the A half exactly
    stage_cap = int(os.environ.get("GNN_STAGE", "9"))
    layer_cap = int(os.environ.get("GNN_LAYERS", "2"))

    # 8 rotating completion sems per queue: call k (per-queue ordinal) uses
    # slot k%8.  With <=8 calls in flight per queue (ring gates), each slot
    # has at most one call in flight, so the threshold 16*(k//8+1) is exact.
    dma_sems = [[nc.alloc_semaphore(f"gdma{q}_{j}") for j in range(8)]
                for q in range(4)]

    with tile.TileContext(nc) as tc:
        with contextlib.ExitStack() as ctx:
            cpool = ctx.enter_context(tc.tile_pool(name="const", bufs=1))
            wp = ctx.enter_context(tc.tile_pool(name="work", bufs=2))
            hp = ctx.enter_context(tc.tile_pool(name="resid", bufs=1))
            pp = ctx.enter_context(tc.tile_pool(name="psA", bufs=1, space="PSUM"))
            pb = ctx.enter_context(tc.tile_pool(name="psB", bufs=2, space="PSUM"))

            # ---- load constants
            wsb = {}
            for k, ext in wext.items():
                t = cpool.tile(list(ext.shape), ext.dtype, tag=k)
                nc.sync.dma_start(out=t[:], in_=ext[:])
                wsb[k] = t
            idx_sb = cpool.tile([BLK, w_idx], I16, tag="idx")
            nc.sync.dma_start(out=idx_sb[:], in_=idx_in[:])
            dl_sb = cpool.tile([BLK, tot_s], BF16, tag="dl")
            nc.sync.dma_start(out=dl_sb[:], in_=dl_in[:])

            hT_res = hp.tile([F, NC_N], F32, tag="hres")
            hT_act = hp.tile([F, NC_N], BF16, tag="hact")
            nc.vector.memset(hT_act[:], 0.0)
            arenaA = hp.tile([BLK, ARENA_S, ROW], BF16, tag="arA")
            arenaB = hp.tile([BLK, ARENA_S, ROW], BF16, tag="arB")
            scr = hp.tile([F, (NC_N + 1) // 2], F32, tag="scr")
            stats = hp.tile([F, 6], F32, tag="stats")
            bnsc = hp.tile([F, 8], F32, tag="bnsc")

            for l in (0, 1)[:layer_cap]:
                d = dims[l]
                # ================= phase A: per-node G rows + x_root =======
                for ci in range(n_chunks):
                    c0 = ci * CHUNK
                    cw = min(CHUNK, NC_N - c0)
                    if l == 0:
                        rhs = wp.tile([IN, CHUNK], BF16, tag="xchunk")
                        nc.sync.dma_start(out=rhs[:, :cw], in_=xT[:, c0:c0 + cw])
                        rhs_ap = rhs[:IN, :cw]
                    else:
                        rhs_ap = hT_act[:F, c0:c0 + cw]

                    ps_jm = pp.tile([H * F, CHUNK], F32, tag="jm", space="PSUM")
                    ps_iq = pp.tile([H * F, CHUNK], F32, tag="iq", space="PSUM")
                    ps_r = pp.tile([F, CHUNK], F32, tag="r", space="PSUM")
                    nc.tensor.matmul(out=ps_jm[:, :cw], lhsT=wsb[f"Wn{l}"][:d, :],
                                     rhs=rhs_ap, start=True, stop=True)
                    nc.tensor.matmul(out=ps_iq[:, :cw], lhsT=wsb[f"Wa{l}"][:d, :],
                                     rhs=rhs_ap, start=True, stop=True)
                    nc.tensor.matmul(out=ps_r[:, :cw], lhsT=wsb[f"Wr{l}"][:d, :],
                                     rhs=rhs_ap, start=True, stop=True)
                    nc.vector.tensor_copy(hT_res[:, c0:c0 + cw], ps_r[:, :cw])

                    jm = wp.tile([H * F, CHUNK], BF16, tag="jm_sb")
                    nc.scalar.activation(jm[:, :cw], ps_jm[:, :cw], AF.Identity)
                    # leaky(x) = max(x, 0.2x)
                    lkjm = wp.tile([H * F, CHUNK], BF16, tag="lkjm")
                    nc.scalar.mul(lkjm[:, :cw], ps_jm[:, :cw], LEAKY)
                    nc.vector.tensor_tensor(out=lkjm[:, :cw], in0=lkjm[:, :cw],
                                            in1=jm[:, :cw], op=OP.max)
                    iq = wp.tile([H * F, CHUNK], BF16, tag="iq_sb")
                    nc.scalar.activation(iq[:, :cw], ps_iq[:, :cw], AF.Identity)
                    lkiq = wp.tile([H * F, CHUNK], BF16, tag="lkiq")
                    nc.scalar.mul(lkiq[:, :cw], ps_iq[:, :cw], LEAKY)
                    nc.vector.tensor_tensor(out=lkiq[:, :cw], in0=lkiq[:, :cw],
                                            in1=iq[:, :cw], op=OP.max)
                    ps_s = pp.tile([H, CHUNK], F32, tag="s", space="PSUM")
                    nc.tensor.matmul(out=ps_s[:, :cw], lhsT=wsb[f"avq{l}"][:],
                                     rhs=lkiq[:, :cw], start=True, stop=False)
                    nc.tensor.matmul(out=ps_s[:, :cw], lhsT=wsb[f"avm{l}"][:],
                                     rhs=lkjm[:, :cw], start=False, stop=True)
                    e_sb = wp.tile([H, CHUNK], BF16, tag="esb")
                    nc.scalar.activation(e_sb[:, :cw], ps_s[:, :cw], AF.Exp)
                    # broadcast E over the per-head 64 features via matmul
                    ps_eb = pp.tile([H * F, CHUNK], F32, tag="iq", space="PSUM")
                    nc.tensor.matmul(out=ps_eb[:, :cw], lhsT=wsb["blkones"][:],
                                     rhs=e_sb[:, :cw], start=True, stop=True)
                    eb = wp.tile([H * F, CHUNK], BF16, tag="eb")
                    nc.scalar.activation(eb[:, :cw], ps_eb[:, :cw], AF.Identity)
                    y = wp.tile([H * F, CHUNK], BF16, tag="y")
                    nc.vector.tensor_tensor(out=y[:, :cw], in0=jm[:, :cw],
                                            in1=eb[:, :cw], op=OP.mult)
                    # write G rows (transpose to node-major)
                    for q in range(0, cw, BLK):
                        qw = min(BLK, cw - q)
                        ps_t = pb.tile([BLK, BLK], BF16, tag="tp", space="PSUM")
                        nc.tensor.transpose(out=ps_t[:qw, :], in_=y[:, q:q + qw],
                                            identity=wsb["identb"][:])
                        ps_e = pb.tile([BLK, BLK], BF16, tag="tp", space="PSUM")
                        nc.tensor.transpose(out=ps_e[:qw, :H], in_=e_sb[:, q:q + qw],
                                            identity=wsb["identb"][:H, :H])
                        gt = wp.tile([BLK, ROW], BF16, tag="gt")
                        nc.vector.tensor_copy(gt[:qw, 0:H * F], ps_t[:qw, :])
                        nc.vector.tensor_copy(gt[:qw, H * F:GVAL], ps_e[:qw, :H])
                        nc.sync.dma_start(
                            out=g_src[l][c0 + q:c0 + q + qw, :],
                            in_=gt[:qw, :])
                    # AllGather half A as soon as its rows are written
                    if ci == a_chunks - 1 and stage_cap >= 2:
                        nc.gpsimd.collective_compute(
                            "AllGather", OP.bypass, replica_groups=groups,
                            ins=[g_src[l][0:ASPLIT, :]], outs=[g_fullA[l][:]])

                if stage_cap < 2:
                    continue
                nc.gpsimd.collective_compute(
                    "AllGather", OP.bypass, replica_groups=groups,
                    ins=[g_src[l][ASPLIT:NC_N, :]], outs=[g_fullB[l][:]])

                # ================= phase B: gather + indicator matmul ======
                if stage_cap < 3:
                    continue
                # Synchronous gather calls are the DEFAULT: the Q7's desc-gen
                # is cheaper in immediate mode (~6.9 vs 8.3 ns/idx) and the
                # DMA overlaps later calls via the 4-queue rotation anyway.
                sync_mode = not os.environ.get("GNN_ASYNC")
                if l == 0:
                    cum_calls = [0, 0, 0, 0]   # per-queue call ordinals
                    prev_prep = [None, None, None, None]
                    prev_trigger = [None, None, None, None]
                    trig_of = {}               # (q, ordinal) -> trigger inst
                    call_ctr = [0]
                emitted = {"A": 0, "B": 0}
                call_trig = {}
                arenas = {"A": arenaA, "B": arenaB}

                def chain(inst, *prevs):
                    deps = bass.InstructionNameOrderedSet()
                    have = False
                    for pv in prevs:
                        if pv is not None:
                            deps.add(pv.ins.name)
                            have = True
                    if have:
                        inst.ins.add_nosync_dependencies_from(deps)

                def emit_call(reg, k):
                    q = call_ctr[0] % 4
                    call_ctr[0] += 1
                    col0 = (0 if reg == "A" else meta["wA"]) + k * (CAP // 16)
                    in_view = g_fullA[l][:] if reg == "A" else g_fullB[l][:]
                    arena = arenas[reg]
                    slot0 = (8 * k) % ARENA_S
                    if sync_mode:
                        nc.gpsimd.dma_gather(
                            out_ap=arena[:, slot0:slot0 + 8, :],
                            in_ap=in_view,
                            idxs_ap=idx_sb[:, col0:col0 + CAP // 16],
                            num_idxs=CAP, num_idxs_reg=CAP,
                            elem_size=ROW, queue_num=q)
                        call_trig[(reg, k)] = None
                        return
                    ordinal = cum_calls[q]
                    slot = ordinal % 8
                    gate = None
                    if ordinal >= GATE_D:
                        # ring-capacity gate: call (ordinal-GATE_D) of this
                        # queue must be fully drained -> at most GATE_D calls
                        # (~260 descs/engine of the ring) in flight per
                        # queue, and the 8 sem slots stay unambiguous
                        og = ordinal - GATE_D
                        gate = nc.gpsimd.wait_ge(dma_sems[q][og % 8],
                                                 16 * (og // 8 + 1))
                        chain(gate, trig_of[(q, og)], prev_prep[q])
                    p = nc.gpsimd.dma_gather(
                        out_ap=arena[:, slot0:slot0 + 8, :],
                        in_ap=in_view,
                        idxs_ap=idx_sb[:, col0:col0 + CAP // 16],
                        num_idxs=CAP, num_idxs_reg=CAP,
                        elem_size=ROW, queue_num=q,
                        prepare_only=True, sem=dma_sems[q][slot])
                    nc._gnn_prep_targets[p.ins.name] = 16 * (ordinal // 8 + 1)
                    chain(p, gate, prev_prep[q])
                    prev_prep[q] = p
                    t = nc.gpsimd.trigger_dma(count=1, queue_num=q)
                    chain(t, p, prev_trigger[q])
                    nc._gnn_prep_trig[p.ins.name] = t.ins
                    prev_trigger[q] = t
                    trig_of[(q, ordinal)] = t
                    call_trig[(reg, k)] = t
                    cum_calls[q] = ordinal + 1

                for b in range(nb):
                    bl = blocks[b]
                    b0 = b * BLK
                    bw = min(BLK, NC_N - b0)
                    while emitted["A"] < bl["needA"]:
                        emit_call("A", emitted["A"])
                        emitted["A"] += 1
                    while emitted["B"] < bl["needB"]:
                        emit_call("B", emitted["B"])
                        emitted["B"] += 1
                    if stage_cap < 4:
                        continue
                    off = bl["dl_off"]
                    n_sub = bl["n_sub"]
                    ind = wp.tile([BLK, s_max * BLK], BF16, tag="ind", bufs=3)
                    nc.vector.tensor_tensor(
                        out=ind[:, 0:n_sub * BLK].rearrange("p (s i) -> p s i", i=BLK),
                        in0=dl_sb[:, off:off + n_sub][:, :, None]
                            .to_broadcast([BLK, n_sub, BLK]),
                        in1=wsb["iota"][:, None, :].to_broadcast([BLK, n_sub, BLK]),
                        op=OP.is_equal)
                    ps_blk = pb.tile([BLK, GVAL], F32, tag="blk", space="PSUM")
                    for j, (reg, s, e0, e1) in enumerate(bl["subs"]):
                        arena = arenas[reg]
                        mm = nc.tensor.matmul(out=ps_blk[:],
                                              lhsT=ind[:, j * BLK:(j + 1) * BLK],
                                              rhs=arena[:, s % ARENA_S, 0:GVAL],
                                              start=(j == 0), stop=(j == n_sub - 1))
                        tg = call_trig.get((reg, s // 8))
                        if mm is not None and tg is not None:
                            # scheduling-order (no-sync) edge: keep stage
                            # consumers after their call's trigger in the PE
                            # stream, else PE head-of-line blocks on data
                            # whose trigger hasn't dispatched yet
                            deps = bass.InstructionNameOrderedSet()
                            deps.add(tg.ins.name)
                            mm.ins.add_nosync_dependencies_from(deps)
                    sb = wp.tile([BLK, GVAL], F32, tag="sbblk")
                    nc.vector.tensor_copy(sb[:], ps_blk[:])
                    rec = wp.tile([BLK, H], F32, tag="rec")
                    nc.vector.tensor_scalar_add(rec[:], sb[:, H * F:GVAL], 1e-30)
                    nc.vector.reciprocal(rec[:], rec[:])
                    agg = wp.tile([BLK, F], F32, tag="agg")
                    tmp = wp.tile([BLK, F], F32, tag="tmp")
                    nc.scalar.activation(agg[:], sb[:, 0:F], AF.Identity,
                                         scale=rec[:, 0:1])
                    nc.scalar.activation(tmp[:], sb[:, F:2 * F], AF.Identity,
                                         scale=rec[:, 1:2])
                    nc.vector.tensor_add(out=agg[:], in0=agg[:], in1=tmp[:])
                    agg_bf = wp.tile([BLK, F], BF16, tag="aggbf")
                    nc.vector.tensor_copy(agg_bf[:], agg[:])
                    ps_t = pb.tile([BLK, BLK], BF16, tag="tp", space="PSUM")
                    nc.tensor.transpose(out=ps_t[:F, :], in_=agg_bf[:, :F],
                                        identity=wsb["identb"][:])
                    nc.vector.tensor_add(out=hT_res[:, b0:b0 + bw],
                                         in0=hT_res[:, b0:b0 + bw],
                                         in1=ps_t[:F, :bw])

                # ================= BatchNorm + ReLU ========================
                if stage_cap < 5:
                    continue
                nc.vector.reduce_sum(out=stats[:, 0:1], in_=hT_res[:, 0:NC_N],
                                     axis=mybir.AxisListType.X)
                half = (NC_N + 1) // 2
                nc.scalar.square(scr[:, 0:half], hT_res[:, 0:half])
                nc.vector.reduce_sum(out=stats[:, 1:2], in_=scr[:, 0:half],
                                     axis=mybir.AxisListType.X)
                nc.scalar.square(scr[:, 0:NC_N - half], hT_res[:, half:NC_N])
                nc.vector.reduce_sum(out=stats[:, 4:5], in_=scr[:, 0:NC_N - half],
                                     axis=mybir.AxisListType.X)
                nc.vector.tensor_add(out=stats[:, 1:2], in0=stats[:, 1:2],
                                     in1=stats[:, 4:5])
                nc.sync.dma_start(out=bn_src[l][:], in_=stats[:, 0:2])
                nc.gpsimd.collective_compute(
                    "AllReduce", OP.add, replica_groups=groups,
                    ins=[bn_src[l][:]], outs=[bn_out[l][:]])
                nc.sync.dma_start(out=stats[:, 2:4], in_=bn_out[l][:])
                nc.scalar.mul(bnsc[:, 0:1], stats[:, 2:3], 1.0 / N)
                nc.scalar.mul(bnsc[:, 1:2], stats[:, 3:4], 1.0 / N)
                nc.vector.tensor_tensor(out=bnsc[:, 2:3], in0=bnsc[:, 0:1],
                                        in1=bnsc[:, 0:1], op=OP.mult)
                nc.vector.tensor_tensor(out=bnsc[:, 2:3], in0=bnsc[:, 1:2],
                                        in1=bnsc[:, 2:3], op=OP.subtract)
                nc.vector.tensor_scalar_add(bnsc[:, 2:3], bnsc[:, 2:3], BN_EPS)
                nc.vector.reciprocal(bnsc[:, 3:4], bnsc[:, 2:3])
                nc.scalar.sqrt(bnsc[:, 4:5], bnsc[:, 3:4])
                nc.vector.tensor_tensor(out=bnsc[:, 5:6], in0=bnsc[:, 4:5],
                                        in1=wsb[f"bn{l}"][:, 0:1], op=OP.mult)
                nc.vector.tensor_tensor(out=bnsc[:, 6:7], in0=bnsc[:, 0:1],
                                        in1=bnsc[:, 5:6], op=OP.mult)
                nc.vector.tensor_tensor(out=bnsc[:, 6:7], in0=wsb[f"bn{l}"][:, 1:2],
                                        in1=bnsc[:, 6:7], op=OP.subtract)
                nc.scalar.activation(hT_act[:, 0:NC_N], hT_res[:, 0:NC_N],
                                     AF.Relu, bias=bnsc[:, 6:7],
                                     scale=bnsc[:, 5:6])

            # ================= head ========================================
            for ci in range(n_chunks):
                c0 = ci * CHUNK
                cw = min(CHUNK, NC_N - c0)
                ps_o = pp.tile([3, CHUNK], F32, tag="s", space="PSUM")
                nc.tensor.matmul(out=ps_o[:, :cw], lhsT=wsb["headW"][:],
                                 rhs=hT_act[:F, c0:c0 + cw], start=True, stop=True)
                osb = wp.tile([3, CHUNK], F32, tag="osb")
                nc.scalar.activation(osb[:, :cw], ps_o[:, :cw],
                                     AF.Identity, bias=wsb["headb"][:, 0:1])
                nc.sync.dma_start(out=out_ext[:, c0:c0 + cw], in_=osb[:, :cw])

    return nc


# ---------------------------------------------------------------- run cache
_CACHE = {}


def _build_inputs(inputs, meta, idx_full, dl_dev):
    w = pack_weights(inputs)
    x = np.asarray(inputs["x"], np.float32)
    in_maps = []
    for c in range(N_CORES):
        m = dict(w)
        m["xT"] = np.ascontiguousarray(
            x[c * NC_N:(c + 1) * NC_N, :].T).astype(BF)
        m["idx"] = np.ascontiguousarray(idx_full[c])
        m["dstloc"] = np.ascontiguousarray(dl_dev[c])
        in_maps.append(m)
    return in_maps


def kernel(**inputs):
    from concourse.bass_utils import run_bass_kernel_spmd

    _install_hookshim()
    edge = np.asarray(inputs["edge_index"])
    key = hashlib.sha1(edge.tobytes()).hexdigest()
    if key not in _CACHE:
        idx_full, dl_dev, meta = preprocess(edge)
        nc = build_program(meta)
        nc.finalize()
        if os.environ.get("GNN_ASYNC"):
            n_remap, n_del, n_xfer = remap_dmasw_waits(nc)
            print(f"remapped DMASW waits on {n_remap} insts, deleted "
                  f"{n_del} IncSwdgeSem, moved {n_xfer} waits to triggers")
        n_fix = legalize_waits(nc)
        if n_fix:
            print(f"legalize_waits fixed {n_fix} instructions post-finalize")
        _CACHE[key] = (idx_full, dl_dev, meta, nc)
    idx_full, dl_dev, meta, nc = _CACHE[key]
    in_maps = _build_inputs(inputs, meta, idx_full, dl_dev)
    res = run_bass_kernel_spmd(
        nc, in_maps, list(range(N_CORES)),
        trace=bool(os.environ.get("GNN_TRACE")))
    if res.exec_time_ns is not None:
        print(f"HW exec time: {res.exec_time_ns} ns")
    out = np.concatenate([res.results[c]["out"] for c in range(N_CORES)],
                         axis=1)  # [3, N]
    return np.ascontiguousarray(out.T).astype(np.float32)
